# revision 13
# baseline (speedup 1.0000x reference)
"""2D B-spline surface kernel for Trainium2 (8 NeuronCores, SPMD).

Problem: out = outer(coefs @ Bx, coefs_2 @ Bt) where Bx/Bt are cubic
B-spline basis matrices (Cox-de Boor) over knots, evaluated at x/t.
Shapes: x[8192], t[8192], knots[132], coefs[128], coefs_2[128],
out[8192, 8192] f32.

Sharding: x is row-sharded across 8 cores (1024 rows each); t / knots /
coefs are replicated. Each core computes its [1024, 8192] block; host
concatenates.

Device algorithm (per core), V1 dense recursion:
  - Layout: basis index i on partitions (128 at degree 3), points on the
    free dim. Points = own x shard (1024) + full t (8192) = 9216, chunked.
  - Degree-0 indicator via two tensor_scalar compares + multiply.
  - Each Cox-de Boor level: u/v affine maps of broadcast points on the
    scalar engine (per-partition scale/bias), B[i+1] partition shift via a
    PE matmul against a subdiagonal permutation matrix (exact: products
    are 1.0 * value), then t1 = u*B, t2 = v*Bshift, B = t1 + t2.
  - Contraction spline = [coefs|coefs_2]^T @ B3 on the TensorEngine.
  - Spline rows are staged to DRAM, re-read as a [128, 8] sx column tile
    and a [128, 8192] broadcast st tile, and the outer product is formed
    by tensor_scalar (DVE) / activation-with-scale (ACT) per 128-row tile,
    then DMA'd out (4 MB contiguous writes).
"""

import numpy as np
from contextlib import ExitStack

import concourse.bass as bass
import concourse.bacc as bacc
import concourse.tile as tile
import concourse.mybir as mybir
from concourse.bass_utils import run_bass_kernel_spmd

AF = mybir.ActivationFunctionType
OP = mybir.AluOpType
DT = mybir.dt.float32

N_CORES = 8
NX = 8192
NT = 8192
NK = 132
NCF = 128
P = 128


def emit_core_program(nc, xs, tpts, knots, coefs, coefs2, out, nxs, nt, w=512):
    """Emit the per-core Tile program. xs..out are DRAM tensor handles."""
    npts = nxs + nt
    assert nxs % w == 0 and nt % w == 0 and npts % w == 0
    nchunks = npts // w
    n_row_tiles = nxs // P

    with tile.TileContext(nc) as tc, ExitStack() as ctx:
        consts = ctx.enter_context(tc.tile_pool(name="consts", bufs=1))
        work = ctx.enter_context(tc.tile_pool(name="work", bufs=2))
        outp = ctx.enter_context(tc.tile_pool(name="outp", bufs=2))
        psum = ctx.enter_context(tc.tile_pool(name="psum", bufs=3, space="PSUM"))
        psum_c = ctx.enter_context(tc.tile_pool(name="psum_c", bufs=2, space="PSUM"))
        dram = ctx.enter_context(tc.tile_pool(name="dram", bufs=1, space="DRAM"))

        # ---- constants ----
        # kcolT[p, k] = knots[p + k], k = 0..4 (overlapping window read)
        kcolT = consts.tile([P, 5], DT)
        ksrc = bass.AP(tensor=knots, offset=0, ap=[[1, P], [1, 5]])
        nc.sync.dma_start(out=kcolT, in_=ksrc)

        # ccol = [coefs | coefs_2] as columns (matmul lhsT, K=128, M=2)
        ccol = consts.tile([P, 2], DT)
        nc.sync.dma_start(
            out=ccol[:, 0:1], in_=coefs.ap().rearrange("(p one) -> p one", one=1)
        )
        nc.sync.dma_start(
            out=ccol[:, 1:2], in_=coefs2.ap().rearrange("(p one) -> p one", one=1)
        )

        # knot-difference reciprocals, masked where the denominator is 0:
        # cols 0..2 = ld_k = knots[i+k]-knots[i], cols 3..5 = rd_k =
        # knots[i+k+1]-knots[i+1], k=1..3
        d6 = consts.tile([P, 6], DT)
        nc.vector.tensor_scalar(
            out=d6[:, 0:3], in0=kcolT[:, 1:4], scalar1=kcolT[:, 0:1],
            scalar2=None, op0=OP.subtract,
        )
        nc.vector.tensor_scalar(
            out=d6[:, 3:6], in0=kcolT[:, 2:5], scalar1=kcolT[:, 1:2],
            scalar2=None, op0=OP.subtract,
        )
        dmask = consts.tile([P, 6], DT)
        nc.vector.tensor_scalar(
            out=dmask, in0=d6, scalar1=0.0, scalar2=None, op0=OP.is_gt
        )
        # safe = max(d, eps): keeps valid denominators bit-exact (no
        # cancellation), makes the reciprocal finite on empty-span rows
        # (those are zeroed by dmask afterwards).
        dsafe = consts.tile([P, 6], DT)
        nc.vector.tensor_scalar(
            out=dsafe, in0=d6, scalar1=1e-6, scalar2=None, op0=OP.max
        )
        ihat = consts.tile([P, 6], DT)
        nc.vector.reciprocal(out=ihat, in_=dsafe)
        nc.vector.tensor_tensor(out=ihat, in0=ihat, in1=dmask, op=OP.mult)

        # v = (knots[i+k+1] - x)*ird = (x - knots[i+k+1]) * (-ird):
        # keep -ird so both u and v use the (subtract, mult) two-scalar form,
        # which avoids cancellation (x - knot is Sterbenz-exact).
        nird = consts.tile([P, 3], DT)
        nc.vector.tensor_scalar(
            out=nird, in0=ihat[:, 3:6], scalar1=-1.0, scalar2=None, op0=OP.mult
        )

        # subdiagonal shift matrix: shm[j, i] = 1 iff j == i+1, so that
        # (shm.T @ B)[i] = B[i+1] (row 127 -> 0)
        ones_t = consts.tile([P, P], DT)
        nc.gpsimd.memset(ones_t, 1.0)
        shm = consts.tile([P, P], DT)
        nc.gpsimd.affine_select(
            out=shm, in_=ones_t, pattern=[[-1, P]], base=-1,
            channel_multiplier=1, compare_op=OP.is_equal, fill=0.0,
        )

        # DRAM scratch for the spline rows: row 0 = sx (first nxs cols),
        # row 1 = st (last nt cols)
        srow_d = dram.tile([2, npts], DT)

        # ---- per-chunk basis recursion + contraction ----
        for ci in range(nchunks):
            g0 = ci * w
            src = xs.ap()[g0 : g0 + w] if g0 < nxs else tpts.ap()[g0 - nxs : g0 - nxs + w]
            xb = work.tile([P, w], DT, tag="xb")
            nc.gpsimd.dma_start(out=xb, in_=src.unsqueeze(0).to_broadcast([P, w]))

            bge = work.tile([P, w], DT, tag="bge")
            nc.vector.tensor_scalar(
                out=bge, in0=xb, scalar1=kcolT[:, 0:1], scalar2=None, op0=OP.is_ge
            )
            blt = work.tile([P, w], DT, tag="blt")
            nc.vector.tensor_scalar(
                out=blt, in0=xb, scalar1=kcolT[:, 1:2], scalar2=None, op0=OP.is_lt
            )
            b = work.tile([P, w], DT, tag="b0")
            nc.vector.tensor_tensor(out=b, in0=bge, in1=blt, op=OP.mult)

            for k in range(1, 4):
                u = work.tile([P, w], DT, tag="u")
                nc.vector.tensor_scalar(
                    out=u, in0=xb, scalar1=kcolT[:, 0:1],
                    scalar2=ihat[:, k - 1 : k], op0=OP.subtract, op1=OP.mult,
                )
                v = work.tile([P, w], DT, tag="v")
                nc.vector.tensor_scalar(
                    out=v, in0=xb, scalar1=kcolT[:, k + 1 : k + 2],
                    scalar2=nird[:, k - 1 : k], op0=OP.subtract, op1=OP.mult,
                )
                bs = psum.tile([P, w], DT, tag="bs")
                nc.tensor.matmul(bs, lhsT=shm, rhs=b, start=True, stop=True)
                t1 = work.tile([P, w], DT, tag="t1")
                nc.vector.tensor_tensor(out=t1, in0=u, in1=b, op=OP.mult)
                t2 = work.tile([P, w], DT, tag="t2")
                nc.vector.tensor_tensor(out=t2, in0=v, in1=bs, op=OP.mult)
                b = work.tile([P, w], DT, tag=f"b{k}")
                nc.gpsimd.tensor_tensor(out=b, in0=t1, in1=t2, op=OP.add)

            ps = psum_c.tile([2, w], DT, tag="contract")
            nc.tensor.matmul(ps, lhsT=ccol, rhs=b, start=True, stop=True)
            stg = work.tile([2, w], DT, tag="stg")
            nc.any.tensor_copy(out=stg, in_=ps)
            nc.sync.dma_start(out=srow_d[:, g0 : g0 + w], in_=stg)

        # ---- outer product ----
        # sx as a column tile: sxcol[p, r] = sx[r*128 + p]
        sxcol = consts.tile([P, n_row_tiles], DT)
        nc.sync.dma_start(
            out=sxcol,
            in_=srow_d[0:1, 0:nxs].rearrange("one (r p) -> p (one r)", p=P),
        )
        # st broadcast to all partitions
        stb = consts.tile([P, nt], DT)
        nc.gpsimd.dma_start(
            out=stb, in_=srow_d[1:2, nxs:npts].to_broadcast([P, nt])
        )

        for r in range(n_row_tiles):
            ot = outp.tile([P, nt], DT, tag="ot")
            if r % 2 == 0:
                nc.vector.tensor_scalar(
                    out=ot, in0=stb, scalar1=sxcol[:, r : r + 1], scalar2=None,
                    op0=OP.mult,
                )
            else:
                nc.scalar.activation(
                    out=ot, in_=stb, func=AF.Copy, scale=sxcol[:, r : r + 1]
                )
            nc.sync.dma_start(out=out.ap()[r * P : (r + 1) * P, :], in_=ot)


def emit_core_program_v2(nc, xs, tpts, knots, coefs, coefs2, out, nxs, nt):
    """Table-based evaluation.

    The spline restricted to span j (j = 0..124, span = [knots[j+3],
    knots[j+4])) is a cubic. We build, on device, a per-span table
    T[j] = [kleft, invh, Ax, Bx, Cx, Dx, At, Bt, Ct, Dt] (the cubic in the
    normalized local coordinate v = (x - kleft)*invh for both coef vectors)
    by evaluating the dense Cox-de Boor recursion at 4 nodes per span
    (v = 0, 1/4, 1/2, 3/4) and applying the exact 4-point interpolation
    matrix. Per-point evaluation then is: step matrix C[i, pt] =
    (x >= knots[i+3]); gathered row = C^T @ deltaT (prefix-sum trick);
    Horner in v. All pointwise work runs in a points-on-partitions layout
    ([128, npts/128]) where it is nearly free.
    """
    npts = nxs + nt
    assert nxs % P == 0 and nt % P == 0
    nch = npts // P            # 128-point gather chunks
    nxch = nxs // P
    n_row_tiles = nxs // P
    GRP = 1024                 # xb broadcast group width
    assert nxs % GRP == 0 and nt % GRP == 0

    # 4-node interpolation matrix (exact, nodes fixed at q/4)
    vand = np.array([[(q / 4.0) ** r for r in range(4)] for q in range(4)], np.float64)
    m4 = np.linalg.inv(vand)  # g = m4 @ sv_nodes

    with tile.TileContext(nc) as tc, ExitStack() as ctx:
        consts = ctx.enter_context(tc.tile_pool(name="consts", bufs=1))
        work = ctx.enter_context(tc.tile_pool(name="work", bufs=2))
        cpool = ctx.enter_context(tc.tile_pool(name="cpool", bufs=3))
        outp = ctx.enter_context(tc.tile_pool(name="outp", bufs=2))
        psum = ctx.enter_context(tc.tile_pool(name="psum", bufs=1, space="PSUM"))
        psum_g = ctx.enter_context(tc.tile_pool(name="psum_g", bufs=4, space="PSUM"))
        dram = ctx.enter_context(tc.tile_pool(name="dram", bufs=1, space="DRAM"))

        # ---- shared constants (same as V1) ----
        kcolT = consts.tile([P, 5], DT)
        nc.sync.dma_start(out=kcolT, in_=bass.AP(tensor=knots, offset=0, ap=[[1, P], [1, 5]]))
        ccol = consts.tile([P, 2], DT)
        nc.sync.dma_start(out=ccol[:, 0:1], in_=coefs.ap().rearrange("(p one) -> p one", one=1))
        nc.sync.dma_start(out=ccol[:, 1:2], in_=coefs2.ap().rearrange("(p one) -> p one", one=1))

        d6 = consts.tile([P, 6], DT)
        nc.vector.tensor_scalar(out=d6[:, 0:3], in0=kcolT[:, 1:4], scalar1=kcolT[:, 0:1], scalar2=None, op0=OP.subtract)
        nc.vector.tensor_scalar(out=d6[:, 3:6], in0=kcolT[:, 2:5], scalar1=kcolT[:, 1:2], scalar2=None, op0=OP.subtract)
        dmask = consts.tile([P, 6], DT)
        nc.vector.tensor_scalar(out=dmask, in0=d6, scalar1=0.0, scalar2=None, op0=OP.is_gt)
        dsafe = consts.tile([P, 6], DT)
        nc.vector.tensor_scalar(out=dsafe, in0=d6, scalar1=1e-6, scalar2=None, op0=OP.max)
        ihat = consts.tile([P, 6], DT)
        nc.vector.reciprocal(out=ihat, in_=dsafe)
        nc.vector.tensor_tensor(out=ihat, in0=ihat, in1=dmask, op=OP.mult)
        nird = consts.tile([P, 3], DT)
        nc.vector.tensor_scalar(out=nird, in0=ihat[:, 3:6], scalar1=-1.0, scalar2=None, op0=OP.mult)

        ones_t = consts.tile([P, P], DT)
        nc.gpsimd.memset(ones_t, 1.0)
        shm = consts.tile([P, P], DT)   # up-shift: (shm.T @ B)[i] = B[i+1]
        nc.gpsimd.affine_select(out=shm, in_=ones_t, pattern=[[-1, P]], base=-1, channel_multiplier=1, compare_op=OP.is_equal, fill=0.0)
        shm2 = consts.tile([P, P], DT)  # down-shift: (shm2.T @ T)[i] = T[i-1]
        nc.gpsimd.affine_select(out=shm2, in_=ones_t, pattern=[[-1, P]], base=1, channel_multiplier=1, compare_op=OP.is_equal, fill=0.0)

        # ---- node construction: y[j, q] = knots[j+3] + (q/4)*h_j ----
        h_col = consts.tile([P, 1], DT)
        nc.vector.tensor_tensor(out=h_col, in0=kcolT[:, 4:5], in1=kcolT[:, 3:4], op=OP.subtract)
        hq_col = consts.tile([P, 1], DT)
        nc.vector.tensor_scalar(out=hq_col, in0=h_col, scalar1=0.25, scalar2=None, op0=OP.mult)
        qi = consts.tile([P, 4], mybir.dt.int32)
        nc.gpsimd.iota(qi, pattern=[[1, 4]], base=0, channel_multiplier=0)
        qf = consts.tile([P, 4], DT)
        nc.vector.tensor_copy(out=qf, in_=qi)
        ynod = consts.tile([P, 4], DT)
        nc.vector.tensor_scalar(out=ynod, in0=qf, scalar1=hq_col, scalar2=kcolT[:, 3:4], op0=OP.mult, op1=OP.add)

        ynod_d = dram.tile([P * 4], DT)
        nc.sync.dma_start(out=ynod_d.rearrange("(j q) -> j q", q=4), in_=ynod)

        # ---- dense CdB at the 512 nodes ----
        wn = P * 4
        xbn = consts.tile([P, wn], DT)
        nc.gpsimd.dma_start(out=xbn, in_=ynod_d.unsqueeze(0).to_broadcast([P, wn]))
        bge = consts.tile([P, wn], DT)
        nc.vector.tensor_scalar(out=bge, in0=xbn, scalar1=kcolT[:, 0:1], scalar2=None, op0=OP.is_ge)
        blt = consts.tile([P, wn], DT)
        nc.vector.tensor_scalar(out=blt, in0=xbn, scalar1=kcolT[:, 1:2], scalar2=None, op0=OP.is_lt)
        bnod = consts.tile([P, wn], DT)
        nc.vector.tensor_tensor(out=bnod, in0=bge, in1=blt, op=OP.mult)
        for k in range(1, 4):
            u = consts.tile([P, wn], DT, tag=f"nu{k}")
            nc.vector.tensor_scalar(out=u, in0=xbn, scalar1=kcolT[:, 0:1], scalar2=ihat[:, k - 1 : k], op0=OP.subtract, op1=OP.mult)
            v = consts.tile([P, wn], DT, tag=f"nv{k}")
            nc.vector.tensor_scalar(out=v, in0=xbn, scalar1=kcolT[:, k + 1 : k + 2], scalar2=nird[:, k - 1 : k], op0=OP.subtract, op1=OP.mult)
            bsp = psum.tile([P, wn], DT, tag="nshift")
            nc.tensor.matmul(bsp, lhsT=shm, rhs=bnod, start=True, stop=True)
            t1 = consts.tile([P, wn], DT, tag=f"nt1{k}")
            nc.vector.tensor_tensor(out=t1, in0=u, in1=bnod, op=OP.mult)
            t2 = consts.tile([P, wn], DT, tag=f"nt2{k}")
            nc.vector.tensor_tensor(out=t2, in0=v, in1=bsp, op=OP.mult)
            bnod = consts.tile([P, wn], DT, tag=f"nb{k}")
            nc.vector.tensor_tensor(out=bnod, in0=t1, in1=t2, op=OP.add)

        svp = psum.tile([2, wn], DT, tag="ncontract")
        nc.tensor.matmul(svp, lhsT=ccol, rhs=bnod, start=True, stop=True)
        sv = consts.tile([2, wn], DT)
        nc.vector.tensor_copy(out=sv, in_=svp)
        svd = dram.tile([2, wn], DT)
        nc.sync.dma_start(out=svd, in_=sv)

        # node values per span: svt[j, c, q] = sv[c, j*4+q]
        svt = consts.tile([P, 2, 4], DT)
        nc.sync.dma_start(
            out=svt,
            in_=bass.AP(tensor=svd.tensor, offset=svd.offset, ap=[[4, P], [wn, 2], [1, 4]]),
        )

        # ---- table: T = [kleft, invh, Ax..Dx, At..Dt] ----
        tt_ = consts.tile([P, 10], DT)
        nc.vector.tensor_copy(out=tt_[:, 0:1], in_=kcolT[:, 3:4])
        hsafe = consts.tile([P, 1], DT)
        nc.vector.tensor_scalar(out=hsafe, in0=h_col, scalar1=1e-6, scalar2=None, op0=OP.max)
        hrec = consts.tile([P, 1], DT)
        nc.vector.reciprocal(out=hrec, in_=hsafe)
        hmask = consts.tile([P, 1], DT)
        nc.vector.tensor_scalar(out=hmask, in0=h_col, scalar1=0.0, scalar2=None, op0=OP.is_gt)
        nc.vector.tensor_tensor(out=tt_[:, 1:2], in0=hrec, in1=hmask, op=OP.mult)

        fit_acc = consts.tile([P, 8], DT)
        for c in range(2):
            for r in range(4):
                col = c * 4 + r
                acc = fit_acc[:, col : col + 1]
                nc.vector.tensor_scalar(out=acc, in0=svt[:, c, 0:1], scalar1=float(m4[r, 0]), scalar2=None, op0=OP.mult)
                for q in range(1, 4):
                    nc.vector.tensor_scalar(
                        out=acc, in0=svt[:, c, q : q + 1], scalar1=float(m4[r, q]),
                        scalar2=acc, op0=OP.mult, op1=OP.add,
                    )
        nc.vector.tensor_copy(out=tt_[:, 2:10], in_=fit_acc)

        # deltaT for the prefix-sum gather
        tshp = psum.tile([P, 10], DT, tag="tshift")
        nc.tensor.matmul(tshp, lhsT=shm2, rhs=tt_, start=True, stop=True)
        dT = consts.tile([P, 10], DT)
        nc.vector.tensor_tensor(out=dT, in0=tt_, in1=tshp, op=OP.subtract)

        # ---- per-point gather: Gall[p, ci*10 + k] ----
        gall = consts.tile([P, nch * 10], DT)
        for gi in range(npts // GRP):
            g0 = gi * GRP
            src = xs.ap()[g0 : g0 + GRP] if g0 < nxs else tpts.ap()[g0 - nxs : g0 - nxs + GRP]
            xbg = work.tile([P, GRP], DT, tag="xbg")
            nc.gpsimd.dma_start(out=xbg, in_=src.unsqueeze(0).to_broadcast([P, GRP]))
            for cj in range(GRP // P):
                ci = gi * (GRP // P) + cj
                cmat = cpool.tile([P, P], DT, tag="cmat")
                nc.vector.tensor_scalar(
                    out=cmat, in0=xbg[:, cj * P : (cj + 1) * P],
                    scalar1=kcolT[:, 3:4], scalar2=None, op0=OP.is_ge,
                )
                psg = psum_g.tile([P, 10], DT, tag="gather")
                nc.tensor.matmul(psg, lhsT=cmat, rhs=dT, start=True, stop=True)
                if ci % 2 == 0:
                    nc.vector.tensor_copy(out=gall[:, ci * 10 : (ci + 1) * 10], in_=psg)
                else:
                    nc.scalar.copy(out=gall[:, ci * 10 : (ci + 1) * 10], in_=psg)

        # ---- pointwise Horner in v = (x - kleft) * invh ----
        xcol = consts.tile([P, nch], DT)
        nc.sync.dma_start(out=xcol[:, 0:nxch], in_=xs.ap().rearrange("(c p) -> p c", p=P))
        nc.sync.dma_start(out=xcol[:, nxch:nch], in_=tpts.ap().rearrange("(c p) -> p c", p=P))

        gv = gall.rearrange("p (c ten) -> p c ten", ten=10)
        wloc = consts.tile([P, nch], DT)
        nc.vector.tensor_tensor(out=wloc, in0=xcol, in1=gv[:, :, 0], op=OP.subtract)
        vloc = consts.tile([P, nch], DT)
        nc.vector.tensor_tensor(out=vloc, in0=wloc, in1=gv[:, :, 1], op=OP.mult)

        sres = consts.tile([P, nch], DT)
        for part, (c0, c1) in enumerate([(0, nxch), (nxch, nch)]):
            base = 2 + part * 4  # Ax..Dx then At..Dt
            vs = vloc[:, c0:c1]
            h1 = consts.tile([P, c1 - c0], DT, tag=f"h1_{part}")
            nc.vector.tensor_tensor(out=h1, in0=gv[:, c0:c1, base + 3], in1=vs, op=OP.mult)
            nc.vector.tensor_tensor(out=h1, in0=h1, in1=gv[:, c0:c1, base + 2], op=OP.add)
            nc.vector.tensor_tensor(out=h1, in0=h1, in1=vs, op=OP.mult)
            nc.vector.tensor_tensor(out=h1, in0=h1, in1=gv[:, c0:c1, base + 1], op=OP.add)
            nc.vector.tensor_tensor(out=h1, in0=h1, in1=vs, op=OP.mult)
            nc.vector.tensor_tensor(out=sres[:, c0:c1], in0=h1, in1=gv[:, c0:c1, base + 0], op=OP.add)

        sxcol = sres[:, 0:nxch]          # [128, 8] column layout, ready for output
        strow_d = dram.tile([nt], DT)
        nc.sync.dma_start(out=strow_d.rearrange("(c p) -> p c", p=P), in_=sres[:, nxch:nch])

        stb = consts.tile([P, nt], DT)
        nc.gpsimd.dma_start(out=stb, in_=strow_d.unsqueeze(0).to_broadcast([P, nt]))

        # ---- outer product + output ----
        for r in range(n_row_tiles):
            ot = outp.tile([P, nt], DT, tag="ot")
            eng = r % 3
            if eng == 0:
                nc.vector.tensor_scalar(out=ot, in0=stb, scalar1=sxcol[:, r : r + 1], scalar2=None, op0=OP.mult)
            elif eng == 1:
                nc.scalar.activation(out=ot, in_=stb, func=AF.Copy, scale=sxcol[:, r : r + 1])
            else:
                nc.gpsimd.tensor_scalar(out=ot, in0=stb, scalar1=sxcol[:, r : r + 1], scalar2=None, op0=OP.mult)
            nc.sync.dma_start(out=out.ap()[r * P : (r + 1) * P, :], in_=ot)


def build_program(nxs=NX // N_CORES, nt=NT, w=512, debug=False, version=1):
    nc = bacc.Bacc("TRN2", target_bir_lowering=False, debug=debug)
    xs = nc.dram_tensor("xs", [nxs], DT, kind="ExternalInput")
    tpts = nc.dram_tensor("t", [nt], DT, kind="ExternalInput")
    knots = nc.dram_tensor("knots", [NK], DT, kind="ExternalInput")
    coefs = nc.dram_tensor("coefs", [NCF], DT, kind="ExternalInput")
    coefs2 = nc.dram_tensor("coefs2", [NCF], DT, kind="ExternalInput")
    out = nc.dram_tensor("out", [nxs, nt], DT, kind="ExternalOutput")
    if version == 2:
        emit_core_program_v2(nc, xs, tpts, knots, coefs, coefs2, out, nxs, nt)
    else:
        emit_core_program(nc, xs, tpts, knots, coefs, coefs2, out, nxs, nt, w)
    nc.compile()
    return nc


VERSION = 1

_NC_CACHE = {}


def _get_program():
    key = VERSION
    if key not in _NC_CACHE:
        _NC_CACHE[key] = build_program(version=VERSION)
    return _NC_CACHE[key]


def kernel(x, t, knots, coefs, coefs_2, _trace=False):
    x = np.ascontiguousarray(np.asarray(x, dtype=np.float32))
    t = np.ascontiguousarray(np.asarray(t, dtype=np.float32))
    knots = np.ascontiguousarray(np.asarray(knots, dtype=np.float32))
    coefs = np.ascontiguousarray(np.asarray(coefs, dtype=np.float32))
    coefs_2 = np.ascontiguousarray(np.asarray(coefs_2, dtype=np.float32))

    nxs = NX // N_CORES
    nc = _get_program()
    in_maps = [
        {
            "xs": np.ascontiguousarray(x[c * nxs : (c + 1) * nxs]),
            "t": t,
            "knots": knots,
            "coefs": coefs,
            "coefs2": coefs_2,
        }
        for c in range(N_CORES)
    ]
    res = run_bass_kernel_spmd(
        nc, in_maps, core_ids=list(range(N_CORES)), trace=_trace
    )
    out = np.concatenate([r["out"] for r in res.results], axis=0)
    if _trace:
        kernel.last_results = res
    return out


# revision 27
# speedup vs baseline: 17064.5045x; 17064.5045x over previous
"""2D B-spline surface kernel for Trainium2 (8 NeuronCores, SPMD).

Problem: out = outer(coefs @ Bx, coefs_2 @ Bt) where Bx/Bt are cubic
B-spline basis matrices (Cox-de Boor) over knots, evaluated at x/t.
Shapes: x[8192], t[8192], knots[132], coefs[128], coefs_2[128],
out[8192, 8192] f32.

Sharding: x is row-sharded across 8 cores (1024 rows each); t / knots /
coefs are replicated. Each core computes its [1024, 8192] block; host
concatenates.

Device algorithm (per core), V1 dense recursion:
  - Layout: basis index i on partitions (128 at degree 3), points on the
    free dim. Points = own x shard (1024) + full t (8192) = 9216, chunked.
  - Degree-0 indicator via two tensor_scalar compares + multiply.
  - Each Cox-de Boor level: u/v affine maps of broadcast points on the
    scalar engine (per-partition scale/bias), B[i+1] partition shift via a
    PE matmul against a subdiagonal permutation matrix (exact: products
    are 1.0 * value), then t1 = u*B, t2 = v*Bshift, B = t1 + t2.
  - Contraction spline = [coefs|coefs_2]^T @ B3 on the TensorEngine.
  - Spline rows are staged to DRAM, re-read as a [128, 8] sx column tile
    and a [128, 8192] broadcast st tile, and the outer product is formed
    by tensor_scalar (DVE) / activation-with-scale (ACT) per 128-row tile,
    then DMA'd out (4 MB contiguous writes).
"""

import numpy as np
from contextlib import ExitStack

import concourse.bass as bass
import concourse.bacc as bacc
import concourse.tile as tile
import concourse.mybir as mybir
from concourse.bass_utils import run_bass_kernel_spmd

AF = mybir.ActivationFunctionType
OP = mybir.AluOpType
DT = mybir.dt.float32

N_CORES = 8
NX = 8192
NT = 8192
NK = 132
NCF = 128
P = 128


def emit_core_program(nc, tc, ctx, xs, tpts, knots, coefs, coefs2, out, nxs, nt, w=512):
    """Emit the per-core Tile program. xs..out are DRAM tensor handles."""
    npts = nxs + nt
    assert nxs % w == 0 and nt % w == 0 and npts % w == 0
    nchunks = npts // w
    n_row_tiles = nxs // P

    if True:
        consts = ctx.enter_context(tc.tile_pool(name="consts", bufs=1))
        work = ctx.enter_context(tc.tile_pool(name="work", bufs=2))
        outp = ctx.enter_context(tc.tile_pool(name="outp", bufs=2))
        psum = ctx.enter_context(tc.tile_pool(name="psum", bufs=3, space="PSUM"))
        psum_c = ctx.enter_context(tc.tile_pool(name="psum_c", bufs=2, space="PSUM"))
        dram = ctx.enter_context(tc.tile_pool(name="dram", bufs=1, space="DRAM"))

        # ---- constants ----
        # kcolT[p, k] = knots[p + k], k = 0..4 (overlapping window read)
        kcolT = consts.tile([P, 5], DT)
        ksrc = bass.AP(tensor=knots, offset=0, ap=[[1, P], [1, 5]])
        nc.sync.dma_start(out=kcolT, in_=ksrc)

        # ccol = [coefs | coefs_2] as columns (matmul lhsT, K=128, M=2)
        ccol = consts.tile([P, 2], DT)
        nc.sync.dma_start(
            out=ccol[:, 0:1], in_=coefs.ap().rearrange("(p one) -> p one", one=1)
        )
        nc.sync.dma_start(
            out=ccol[:, 1:2], in_=coefs2.ap().rearrange("(p one) -> p one", one=1)
        )

        # knot-difference reciprocals, masked where the denominator is 0:
        # cols 0..2 = ld_k = knots[i+k]-knots[i], cols 3..5 = rd_k =
        # knots[i+k+1]-knots[i+1], k=1..3
        d6 = consts.tile([P, 6], DT)
        nc.vector.tensor_scalar(
            out=d6[:, 0:3], in0=kcolT[:, 1:4], scalar1=kcolT[:, 0:1],
            scalar2=None, op0=OP.subtract,
        )
        nc.vector.tensor_scalar(
            out=d6[:, 3:6], in0=kcolT[:, 2:5], scalar1=kcolT[:, 1:2],
            scalar2=None, op0=OP.subtract,
        )
        dmask = consts.tile([P, 6], DT)
        nc.vector.tensor_scalar(
            out=dmask, in0=d6, scalar1=0.0, scalar2=None, op0=OP.is_gt
        )
        # safe = max(d, eps): keeps valid denominators bit-exact (no
        # cancellation), makes the reciprocal finite on empty-span rows
        # (those are zeroed by dmask afterwards).
        dsafe = consts.tile([P, 6], DT)
        nc.vector.tensor_scalar(
            out=dsafe, in0=d6, scalar1=1e-6, scalar2=None, op0=OP.max
        )
        ihat = consts.tile([P, 6], DT)
        nc.vector.reciprocal(out=ihat, in_=dsafe)
        nc.vector.tensor_tensor(out=ihat, in0=ihat, in1=dmask, op=OP.mult)

        # v = (knots[i+k+1] - x)*ird = (x - knots[i+k+1]) * (-ird):
        # keep -ird so both u and v use the (subtract, mult) two-scalar form,
        # which avoids cancellation (x - knot is Sterbenz-exact).
        nird = consts.tile([P, 3], DT)
        nc.vector.tensor_scalar(
            out=nird, in0=ihat[:, 3:6], scalar1=-1.0, scalar2=None, op0=OP.mult
        )

        # subdiagonal shift matrix: shm[j, i] = 1 iff j == i+1, so that
        # (shm.T @ B)[i] = B[i+1] (row 127 -> 0)
        ones_t = consts.tile([P, P], DT)
        nc.gpsimd.memset(ones_t, 1.0)
        shm = consts.tile([P, P], DT)
        nc.gpsimd.affine_select(
            out=shm, in_=ones_t, pattern=[[-1, P]], base=-1,
            channel_multiplier=1, compare_op=OP.is_equal, fill=0.0,
        )

        # DRAM scratch for the spline rows: row 0 = sx (first nxs cols),
        # row 1 = st (last nt cols)
        srow_d = dram.tile([2, npts], DT)

        # ---- per-chunk basis recursion + contraction ----
        for ci in range(nchunks):
            g0 = ci * w
            src = xs.ap()[g0 : g0 + w] if g0 < nxs else tpts.ap()[g0 - nxs : g0 - nxs + w]
            xb = work.tile([P, w], DT, tag="xb")
            nc.gpsimd.dma_start(out=xb, in_=src.unsqueeze(0).to_broadcast([P, w]))

            bge = work.tile([P, w], DT, tag="bge")
            nc.vector.tensor_scalar(
                out=bge, in0=xb, scalar1=kcolT[:, 0:1], scalar2=None, op0=OP.is_ge
            )
            blt = work.tile([P, w], DT, tag="blt")
            nc.vector.tensor_scalar(
                out=blt, in0=xb, scalar1=kcolT[:, 1:2], scalar2=None, op0=OP.is_lt
            )
            b = work.tile([P, w], DT, tag="b0")
            nc.vector.tensor_tensor(out=b, in0=bge, in1=blt, op=OP.mult)

            for k in range(1, 4):
                u = work.tile([P, w], DT, tag="u")
                nc.vector.tensor_scalar(
                    out=u, in0=xb, scalar1=kcolT[:, 0:1],
                    scalar2=ihat[:, k - 1 : k], op0=OP.subtract, op1=OP.mult,
                )
                v = work.tile([P, w], DT, tag="v")
                nc.vector.tensor_scalar(
                    out=v, in0=xb, scalar1=kcolT[:, k + 1 : k + 2],
                    scalar2=nird[:, k - 1 : k], op0=OP.subtract, op1=OP.mult,
                )
                bs = psum.tile([P, w], DT, tag="bs")
                nc.tensor.matmul(bs, lhsT=shm, rhs=b, start=True, stop=True)
                t1 = work.tile([P, w], DT, tag="t1")
                nc.vector.tensor_tensor(out=t1, in0=u, in1=b, op=OP.mult)
                t2 = work.tile([P, w], DT, tag="t2")
                nc.vector.tensor_tensor(out=t2, in0=v, in1=bs, op=OP.mult)
                b = work.tile([P, w], DT, tag=f"b{k}")
                nc.gpsimd.tensor_tensor(out=b, in0=t1, in1=t2, op=OP.add)

            ps = psum_c.tile([2, w], DT, tag="contract")
            nc.tensor.matmul(ps, lhsT=ccol, rhs=b, start=True, stop=True)
            stg = work.tile([2, w], DT, tag="stg")
            nc.any.tensor_copy(out=stg, in_=ps)
            nc.sync.dma_start(out=srow_d[:, g0 : g0 + w], in_=stg)

        # ---- outer product ----
        # sx as a column tile: sxcol[p, r] = sx[r*128 + p]
        sxcol = consts.tile([P, n_row_tiles], DT)
        nc.sync.dma_start(
            out=sxcol,
            in_=srow_d[0:1, 0:nxs].rearrange("one (r p) -> p (one r)", p=P),
        )
        # st broadcast to all partitions
        stb = consts.tile([P, nt], DT)
        nc.gpsimd.dma_start(
            out=stb, in_=srow_d[1:2, nxs:npts].to_broadcast([P, nt])
        )

        for r in range(n_row_tiles):
            ot = outp.tile([P, nt], DT, tag="ot")
            if r % 2 == 0:
                nc.vector.tensor_scalar(
                    out=ot, in0=stb, scalar1=sxcol[:, r : r + 1], scalar2=None,
                    op0=OP.mult,
                )
            else:
                nc.scalar.activation(
                    out=ot, in_=stb, func=AF.Copy, scale=sxcol[:, r : r + 1]
                )
            nc.sync.dma_start(out=out.ap()[r * P : (r + 1) * P, :], in_=ot)


def emit_core_program_v2(nc, tc, ctx, xs, tpts, knots, coefs, coefs2, out, nxs, nt):
    """Table-based evaluation.

    The spline restricted to span j (j = 0..124, span = [knots[j+3],
    knots[j+4])) is a cubic. We build, on device, a per-span table
    T[j] = [kleft, invh, Ax, Bx, Cx, Dx, At, Bt, Ct, Dt] (the cubic in the
    normalized local coordinate v = (x - kleft)*invh for both coef vectors)
    by evaluating the dense Cox-de Boor recursion at 4 nodes per span
    (v = 0, 1/4, 1/2, 3/4) and applying the exact 4-point interpolation
    matrix. Per-point evaluation then is: step matrix C[i, pt] =
    (x >= knots[i+3]); gathered row = C^T @ deltaT (prefix-sum trick);
    Horner in v. All pointwise work runs in a points-on-partitions layout
    ([128, npts/128]) where it is nearly free.
    """
    npts = nxs + nt
    assert nxs % P == 0 and nt % P == 0
    nch = npts // P            # 128-point gather chunks
    nxch = nxs // P
    n_row_tiles = nxs // P
    GRP = 1024                 # xb broadcast group width
    assert nxs % GRP == 0 and nt % GRP == 0

    # Lagrange nodes in normalized local coordinate v, spanning [0, 1).
    # v=1 itself is excluded (the dense-CdB node evaluation at the right
    # knot of the LAST span would return 0, not the left limit), so the
    # last node sits just inside.
    NODES = [0.0, 1.0 / 3.0, 2.0 / 3.0, 1.0 - 1.0 / 4096.0]
    # barycentric-style scale: c_q = 1 / prod_{r != q} (v_q - v_r)
    LAGC = []
    for q in range(4):
        prod = 1.0
        for r in range(4):
            if r != q:
                prod *= NODES[q] - NODES[r]
        LAGC.append(1.0 / prod)

    if True:
        consts = ctx.enter_context(tc.tile_pool(name="consts", bufs=1))
        work = ctx.enter_context(tc.tile_pool(name="work", bufs=2))
        cpool = ctx.enter_context(tc.tile_pool(name="cpool", bufs=3))
        outp = ctx.enter_context(tc.tile_pool(name="outp", bufs=2))
        psum = ctx.enter_context(tc.tile_pool(name="psum", bufs=1, space="PSUM"))
        psum_g = ctx.enter_context(tc.tile_pool(name="psum_g", bufs=4, space="PSUM"))
        dram = ctx.enter_context(tc.tile_pool(name="dram", bufs=1, space="DRAM"))

        # ---- shared constants (same as V1) ----
        kcolT = consts.tile([P, 5], DT)
        nc.sync.dma_start(out=kcolT, in_=bass.AP(tensor=knots, offset=0, ap=[[1, P], [1, 5]]))
        ccol = consts.tile([P, 2], DT)
        nc.sync.dma_start(out=ccol[:, 0:1], in_=coefs.ap().rearrange("(p one) -> p one", one=1))
        nc.sync.dma_start(out=ccol[:, 1:2], in_=coefs2.ap().rearrange("(p one) -> p one", one=1))

        d6 = consts.tile([P, 6], DT)
        nc.vector.tensor_scalar(out=d6[:, 0:3], in0=kcolT[:, 1:4], scalar1=kcolT[:, 0:1], scalar2=None, op0=OP.subtract)
        nc.vector.tensor_scalar(out=d6[:, 3:6], in0=kcolT[:, 2:5], scalar1=kcolT[:, 1:2], scalar2=None, op0=OP.subtract)
        dmask = consts.tile([P, 6], DT)
        nc.vector.tensor_scalar(out=dmask, in0=d6, scalar1=0.0, scalar2=None, op0=OP.is_gt)
        dsafe = consts.tile([P, 6], DT)
        nc.vector.tensor_scalar(out=dsafe, in0=d6, scalar1=1e-6, scalar2=None, op0=OP.max)
        ihat = consts.tile([P, 6], DT)
        nc.vector.reciprocal(out=ihat, in_=dsafe)
        nc.vector.tensor_tensor(out=ihat, in0=ihat, in1=dmask, op=OP.mult)
        nird = consts.tile([P, 3], DT)
        nc.vector.tensor_scalar(out=nird, in0=ihat[:, 3:6], scalar1=-1.0, scalar2=None, op0=OP.mult)

        ones_t = consts.tile([P, P], DT)
        nc.gpsimd.memset(ones_t, 1.0)
        shm = consts.tile([P, P], DT)   # up-shift: (shm.T @ B)[i] = B[i+1]
        nc.gpsimd.affine_select(out=shm, in_=ones_t, pattern=[[-1, P]], base=-1, channel_multiplier=1, compare_op=OP.is_equal, fill=0.0)


        # ---- node construction: y[j, q] = knots[j+3] + v_q * h_j ----
        h_col = consts.tile([P, 1], DT)
        nc.vector.tensor_tensor(out=h_col, in0=kcolT[:, 4:5], in1=kcolT[:, 3:4], op=OP.subtract)
        ynod = consts.tile([P, 4], DT)
        for q in range(4):
            nc.vector.tensor_scalar(
                out=ynod[:, q : q + 1], in0=h_col, scalar1=float(NODES[q]),
                scalar2=kcolT[:, 3:4], op0=OP.mult, op1=OP.add,
            )

        ynod_d = dram.tile([P * 4], DT)
        nc.sync.dma_start(out=ynod_d.rearrange("(j q) -> j q", q=4), in_=ynod)

        # ---- dense CdB at the 512 nodes ----
        wn = P * 4
        xbn = consts.tile([P, wn], DT)
        nc.gpsimd.dma_start(out=xbn, in_=ynod_d.unsqueeze(0).to_broadcast([P, wn]))
        bge = consts.tile([P, wn], DT)
        nc.vector.tensor_scalar(out=bge, in0=xbn, scalar1=kcolT[:, 0:1], scalar2=None, op0=OP.is_ge)
        blt = consts.tile([P, wn], DT)
        nc.vector.tensor_scalar(out=blt, in0=xbn, scalar1=kcolT[:, 1:2], scalar2=None, op0=OP.is_lt)
        bnod = consts.tile([P, wn], DT)
        nc.vector.tensor_tensor(out=bnod, in0=bge, in1=blt, op=OP.mult)
        for k in range(1, 4):
            u = consts.tile([P, wn], DT, tag=f"nu{k}")
            nc.vector.tensor_scalar(out=u, in0=xbn, scalar1=kcolT[:, 0:1], scalar2=ihat[:, k - 1 : k], op0=OP.subtract, op1=OP.mult)
            v = consts.tile([P, wn], DT, tag=f"nv{k}")
            nc.vector.tensor_scalar(out=v, in0=xbn, scalar1=kcolT[:, k + 1 : k + 2], scalar2=nird[:, k - 1 : k], op0=OP.subtract, op1=OP.mult)
            bsp = psum.tile([P, wn], DT, tag="nshift")
            nc.tensor.matmul(bsp, lhsT=shm, rhs=bnod, start=True, stop=True)
            t1 = consts.tile([P, wn], DT, tag=f"nt1{k}")
            nc.vector.tensor_tensor(out=t1, in0=u, in1=bnod, op=OP.mult)
            t2 = consts.tile([P, wn], DT, tag=f"nt2{k}")
            nc.vector.tensor_tensor(out=t2, in0=v, in1=bsp, op=OP.mult)
            bnod = consts.tile([P, wn], DT, tag=f"nb{k}")
            nc.vector.tensor_tensor(out=bnod, in0=t1, in1=t2, op=OP.add)

        svp = psum.tile([2, wn], DT, tag="ncontract")
        nc.tensor.matmul(svp, lhsT=ccol, rhs=bnod, start=True, stop=True)
        sv = consts.tile([2, wn], DT)
        nc.vector.tensor_copy(out=sv, in_=svp)
        svd = dram.tile([2, wn], DT)
        nc.sync.dma_start(out=svd, in_=sv)

        # node values per span: svt[j, c, q] = sv[c, j*4+q]
        svt = consts.tile([P, 2, 4], DT)
        nc.sync.dma_start(
            out=svt,
            in_=bass.AP(tensor=svd.tensor, offset=svd.offset, ap=[[4, P], [wn, 2], [1, 4]]),
        )

        # ---- table: T = [kleft, invh, c_q*svx_q (q=0..3), c_q*svt_q] ----
        tt_ = consts.tile([P, 10], DT)
        nc.vector.tensor_copy(out=tt_[:, 0:1], in_=kcolT[:, 3:4])
        hsafe = consts.tile([P, 1], DT)
        nc.vector.tensor_scalar(out=hsafe, in0=h_col, scalar1=1e-6, scalar2=None, op0=OP.max)
        hrec = consts.tile([P, 1], DT)
        nc.vector.reciprocal(out=hrec, in_=hsafe)
        hmask = consts.tile([P, 1], DT)
        nc.vector.tensor_scalar(out=hmask, in0=h_col, scalar1=0.0, scalar2=None, op0=OP.is_gt)
        nc.vector.tensor_tensor(out=tt_[:, 1:2], in0=hrec, in1=hmask, op=OP.mult)

        for c in range(2):
            for q in range(4):
                nc.vector.tensor_scalar(
                    out=tt_[:, 2 + c * 4 + q : 3 + c * 4 + q],
                    in0=svt[:, c, q : q + 1], scalar1=float(LAGC[q]),
                    scalar2=None, op0=OP.mult,
                )

        # ---- per-point gather via exact one-hot (no prefix-sum rounding):
        # onehot[i, pt] = (knots[i+3] <= x_pt) * (x_pt < knots[i+4]),
        # gathered row = onehot^T @ T (single 1.0 * T[span] product).
        gall = consts.tile([P, nch * 10], DT)
        for gi in range(npts // GRP):
            g0 = gi * GRP
            src = xs.ap()[g0 : g0 + GRP] if g0 < nxs else tpts.ap()[g0 - nxs : g0 - nxs + GRP]
            xbg = work.tile([P, GRP], DT, tag="xbg")
            nc.gpsimd.dma_start(out=xbg, in_=src.unsqueeze(0).to_broadcast([P, GRP]))
            for cj in range(GRP // P):
                ci = gi * (GRP // P) + cj
                xbc = xbg[:, cj * P : (cj + 1) * P]
                cge = cpool.tile([P, P], DT, tag="cge")
                nc.vector.tensor_scalar(
                    out=cge, in0=xbc, scalar1=kcolT[:, 3:4], scalar2=None, op0=OP.is_ge
                )
                clt = cpool.tile([P, P], DT, tag="clt")
                nc.vector.tensor_scalar(
                    out=clt, in0=xbc, scalar1=kcolT[:, 4:5], scalar2=None, op0=OP.is_lt
                )
                cmat = cpool.tile([P, P], DT, tag="cmat")
                nc.vector.tensor_tensor(out=cmat, in0=cge, in1=clt, op=OP.mult)
                psg = psum_g.tile([P, 10], DT, tag="gather")
                nc.tensor.matmul(psg, lhsT=cmat, rhs=tt_, start=True, stop=True)
                if ci % 2 == 0:
                    nc.vector.tensor_copy(out=gall[:, ci * 10 : (ci + 1) * 10], in_=psg)
                else:
                    nc.scalar.copy(out=gall[:, ci * 10 : (ci + 1) * 10], in_=psg)

        # ---- pointwise Horner in v = (x - kleft) * invh ----
        xcol = consts.tile([P, nch], DT)
        nc.sync.dma_start(out=xcol[:, 0:nxch], in_=xs.ap().rearrange("(c p) -> p c", p=P))
        nc.sync.dma_start(out=xcol[:, nxch:nch], in_=tpts.ap().rearrange("(c p) -> p c", p=P))

        gv = gall.rearrange("p (c ten) -> p c ten", ten=10)
        wloc = consts.tile([P, nch], DT)
        nc.vector.tensor_tensor(out=wloc, in0=xcol, in1=gv[:, :, 0], op=OP.subtract)
        vloc = consts.tile([P, nch], DT)
        nc.vector.tensor_tensor(out=vloc, in0=wloc, in1=gv[:, :, 1], op=OP.mult)

        # Lagrange basis products: d_q = v - v_q; L0' = d1*d2*d3, etc.
        dq = [consts.tile([P, nch], DT, tag=f"dq{q}", name=f"dq{q}") for q in range(4)]
        for q in range(4):
            nc.vector.tensor_scalar(
                out=dq[q], in0=vloc, scalar1=float(NODES[q]), scalar2=None,
                op0=OP.subtract,
            )
        d01 = consts.tile([P, nch], DT)
        nc.vector.tensor_tensor(out=d01, in0=dq[0], in1=dq[1], op=OP.mult)
        d23 = consts.tile([P, nch], DT)
        nc.vector.tensor_tensor(out=d23, in0=dq[2], in1=dq[3], op=OP.mult)
        lag = [consts.tile([P, nch], DT, tag=f"lag{q}", name=f"lag{q}") for q in range(4)]
        nc.vector.tensor_tensor(out=lag[0], in0=dq[1], in1=d23, op=OP.mult)
        nc.vector.tensor_tensor(out=lag[1], in0=dq[0], in1=d23, op=OP.mult)
        nc.vector.tensor_tensor(out=lag[2], in0=dq[3], in1=d01, op=OP.mult)
        nc.vector.tensor_tensor(out=lag[3], in0=dq[2], in1=d01, op=OP.mult)

        # s = sum_q L_q' * (c_q * sv_q)  (scale folded into the table)
        sres = consts.tile([P, nch], DT)
        for part, (c0, c1) in enumerate([(0, nxch), (nxch, nch)]):
            base = 2 + part * 4
            acc = consts.tile([P, c1 - c0], DT, tag=f"acc{part}")
            t0_ = consts.tile([P, c1 - c0], DT, tag=f"t0_{part}")
            nc.vector.tensor_tensor(out=acc, in0=lag[0][:, c0:c1], in1=gv[:, c0:c1, base + 0], op=OP.mult)
            for q in range(1, 4):
                nc.vector.tensor_tensor(out=t0_, in0=lag[q][:, c0:c1], in1=gv[:, c0:c1, base + q], op=OP.mult)
                nc.vector.tensor_tensor(out=acc, in0=acc, in1=t0_, op=OP.add)
            nc.vector.tensor_copy(out=sres[:, c0:c1], in_=acc)

        sxcol = sres[:, 0:nxch]          # [128, 8] column layout, ready for output
        strow_d = dram.tile([nt], DT)
        nc.sync.dma_start(out=strow_d.rearrange("(c p) -> p c", p=P), in_=sres[:, nxch:nch])

        stb = consts.tile([P, nt], DT)
        nc.gpsimd.dma_start(out=stb, in_=strow_d.unsqueeze(0).to_broadcast([P, nt]))

        # ---- outer product + output ----
        for r in range(n_row_tiles):
            ot = outp.tile([P, nt], DT, tag="ot")
            eng = r % 3
            if eng == 0:
                nc.vector.tensor_scalar(out=ot, in0=stb, scalar1=sxcol[:, r : r + 1], scalar2=None, op0=OP.mult)
            elif eng == 1:
                nc.scalar.activation(out=ot, in_=stb, func=AF.Copy, scale=sxcol[:, r : r + 1])
            else:
                nc.gpsimd.tensor_scalar(out=ot, in0=stb, scalar1=sxcol[:, r : r + 1], scalar2=None, op0=OP.mult)
            nc.sync.dma_start(out=out.ap()[r * P : (r + 1) * P, :], in_=ot)


def build_program(nxs=NX // N_CORES, nt=NT, w=512, debug=False, version=1,
                  unroll=1):
    """Build the per-core Bacc program. unroll>1 emits the body N times
    (used for marginal-difference timing)."""
    nc = bacc.Bacc("TRN2", target_bir_lowering=False, debug=debug)
    xs = nc.dram_tensor("xs", [nxs], DT, kind="ExternalInput")
    tpts = nc.dram_tensor("t", [nt], DT, kind="ExternalInput")
    knots = nc.dram_tensor("knots", [NK], DT, kind="ExternalInput")
    coefs = nc.dram_tensor("coefs", [NCF], DT, kind="ExternalInput")
    coefs2 = nc.dram_tensor("coefs2", [NCF], DT, kind="ExternalInput")
    out = nc.dram_tensor("out", [nxs, nt], DT, kind="ExternalOutput")

    emit = emit_core_program_v2 if version == 2 else emit_core_program
    kw = {} if version == 2 else {"w": w}

    with tile.TileContext(nc) as tc, ExitStack() as ctx:
        if unroll == 1:
            emit(nc, tc, ctx, xs, tpts, knots, coefs, coefs2, out, nxs, nt, **kw)
        else:
            for _ in range(unroll):
                with ExitStack() as bctx:
                    emit(nc, tc, bctx, xs, tpts, knots, coefs, coefs2, out, nxs, nt, **kw)
    nc.compile()
    return nc


VERSION = 2

_RUNNER_CACHE = {}


class _Runner:
    """Persistent PJRT runner: jit-compiled once, reusable across calls."""

    def __init__(self, version, unroll=1):
        import jax
        from jax.sharding import Mesh, PartitionSpec
        from jax.experimental.shard_map import shard_map
        from concourse import bass2jax, mybir as mb

        bass2jax.install_neuronx_cc_hook()
        nc = build_program(version=version, unroll=unroll)
        self.nc = nc

        partition_name = (
            nc.partition_id_tensor.name if nc.partition_id_tensor else None
        )
        in_names, out_names, out_avals, zero_shapes = [], [], [], []
        for alloc in nc.m.functions[0].allocations:
            if not isinstance(alloc, mb.MemoryLocationSet):
                continue
            name = alloc.memorylocations[0].name
            if alloc.kind == "ExternalInput":
                if name != partition_name:
                    in_names.append(name)
            elif alloc.kind == "ExternalOutput":
                shape = tuple(alloc.tensor_shape)
                dtype = mb.dt.np(alloc.dtype)
                out_names.append(name)
                out_avals.append(jax.core.ShapedArray(shape, dtype))
                zero_shapes.append((shape, dtype))
        self.in_names = list(in_names)
        self.out_names = out_names
        self.zero_shapes = zero_shapes
        n_params = len(in_names)
        all_in_names = in_names + out_names
        if partition_name is not None:
            all_in_names = all_in_names + [partition_name]

        def _body(*args):
            operands = list(args)
            if partition_name is not None:
                operands.append(bass2jax.partition_id_tensor())
            outs = bass2jax._bass_exec_p.bind(
                *operands,
                out_avals=tuple(out_avals),
                in_names=tuple(all_in_names),
                out_names=tuple(out_names),
                lowering_input_output_aliases=(),
                sim_require_finite=True,
                sim_require_nnan=True,
                nc=nc,
            )
            return tuple(outs)

        devices = jax.devices()[:N_CORES]
        self.mesh = Mesh(np.asarray(devices), ("core",))
        n_outs = len(out_names)
        in_specs = (PartitionSpec("core"),) * (n_params + n_outs)
        out_specs = (PartitionSpec("core"),) * n_outs
        self.donate = tuple(range(n_params, n_params + n_outs))
        self.sharded = jax.jit(
            shard_map(_body, mesh=self.mesh, in_specs=in_specs,
                      out_specs=out_specs, check_rep=False),
            donate_argnums=self.donate,
            keep_unused=True,
        )
        self._jax = jax
        self._P = PartitionSpec

    def make_zeros(self):
        """Fresh donated output buffers, created on-device."""
        import jax
        import jax.numpy as jnp
        from jax.sharding import NamedSharding

        outs = []
        for shape, dtype in self.zero_shapes:
            gshape = (shape[0] * N_CORES,) + tuple(shape[1:])
            sharding = NamedSharding(self.mesh, self._P("core"))
            outs.append(jax.jit(
                lambda s=gshape, d=dtype: jnp.zeros(s, d),
                out_shardings=sharding)())
        return outs

    def run_device(self, concat_ins, zeros):
        """Returns device arrays (not transferred)."""
        return self.sharded(*concat_ins, *zeros)

    def run(self, in_maps):
        concat_ins = [
            np.concatenate([np.asarray(m[name]) for m in in_maps], axis=0)
            for name in self.in_names
        ]
        outs = self.run_device(concat_ins, self.make_zeros())
        return [np.asarray(o) for o in outs]


def _get_runner(version=None, unroll=1):
    key = (version or VERSION, unroll)
    if key not in _RUNNER_CACHE:
        _RUNNER_CACHE[key] = _Runner(key[0], unroll=key[1])
    return _RUNNER_CACHE[key]


def kernel(x, t, knots, coefs, coefs_2):
    x = np.ascontiguousarray(np.asarray(x, dtype=np.float32))
    t = np.ascontiguousarray(np.asarray(t, dtype=np.float32))
    knots = np.ascontiguousarray(np.asarray(knots, dtype=np.float32))
    coefs = np.ascontiguousarray(np.asarray(coefs, dtype=np.float32))
    coefs_2 = np.ascontiguousarray(np.asarray(coefs_2, dtype=np.float32))

    nxs = NX // N_CORES
    runner = _get_runner()
    in_maps = [
        {
            "xs": x[c * nxs : (c + 1) * nxs],
            "t": t,
            "knots": knots,
            "coefs": coefs,
            "coefs2": coefs_2,
        }
        for c in range(N_CORES)
    ]
    outs = runner.run(in_maps)
    return outs[runner.out_names.index("out")].reshape(NX, NT)


# revision 29
# speedup vs baseline: 83729.8383x; 4.9067x over previous
"""2D B-spline surface kernel for Trainium2 (8 NeuronCores, SPMD).

Problem: out = outer(coefs @ Bx, coefs_2 @ Bt) where Bx/Bt are cubic
B-spline basis matrices (Cox-de Boor) over knots, evaluated at x/t.
Shapes: x[8192], t[8192], knots[132], coefs[128], coefs_2[128],
out[8192, 8192] f32.

Sharding: x is row-sharded across 8 cores (1024 rows each); t / knots /
coefs are replicated. Each core computes its [1024, 8192] block; host
concatenates.

Device algorithm (per core), V1 dense recursion:
  - Layout: basis index i on partitions (128 at degree 3), points on the
    free dim. Points = own x shard (1024) + full t (8192) = 9216, chunked.
  - Degree-0 indicator via two tensor_scalar compares + multiply.
  - Each Cox-de Boor level: u/v affine maps of broadcast points on the
    scalar engine (per-partition scale/bias), B[i+1] partition shift via a
    PE matmul against a subdiagonal permutation matrix (exact: products
    are 1.0 * value), then t1 = u*B, t2 = v*Bshift, B = t1 + t2.
  - Contraction spline = [coefs|coefs_2]^T @ B3 on the TensorEngine.
  - Spline rows are staged to DRAM, re-read as a [128, 8] sx column tile
    and a [128, 8192] broadcast st tile, and the outer product is formed
    by tensor_scalar (DVE) / activation-with-scale (ACT) per 128-row tile,
    then DMA'd out (4 MB contiguous writes).
"""

import numpy as np
from contextlib import ExitStack

import concourse.bass as bass
import concourse.bacc as bacc
import concourse.tile as tile
import concourse.mybir as mybir
from concourse.bass_utils import run_bass_kernel_spmd

AF = mybir.ActivationFunctionType
OP = mybir.AluOpType
DT = mybir.dt.float32

N_CORES = 8
NX = 8192
NT = 8192
NK = 132
NCF = 128
P = 128


def emit_core_program(nc, tc, ctx, xs, tpts, knots, coefs, coefs2, out, nxs, nt, w=512):
    """Emit the per-core Tile program. xs..out are DRAM tensor handles."""
    npts = nxs + nt
    assert nxs % w == 0 and nt % w == 0 and npts % w == 0
    nchunks = npts // w
    n_row_tiles = nxs // P

    if True:
        consts = ctx.enter_context(tc.tile_pool(name="consts", bufs=1))
        work = ctx.enter_context(tc.tile_pool(name="work", bufs=2))
        outp = ctx.enter_context(tc.tile_pool(name="outp", bufs=2))
        psum = ctx.enter_context(tc.tile_pool(name="psum", bufs=3, space="PSUM"))
        psum_c = ctx.enter_context(tc.tile_pool(name="psum_c", bufs=2, space="PSUM"))
        dram = ctx.enter_context(tc.tile_pool(name="dram", bufs=1, space="DRAM"))

        # ---- constants ----
        # kcolT[p, k] = knots[p + k], k = 0..4 (overlapping window read)
        kcolT = consts.tile([P, 5], DT)
        ksrc = bass.AP(tensor=knots, offset=0, ap=[[1, P], [1, 5]])
        nc.sync.dma_start(out=kcolT, in_=ksrc)

        # ccol = [coefs | coefs_2] as columns (matmul lhsT, K=128, M=2)
        ccol = consts.tile([P, 2], DT)
        nc.sync.dma_start(
            out=ccol[:, 0:1], in_=coefs.ap().rearrange("(p one) -> p one", one=1)
        )
        nc.sync.dma_start(
            out=ccol[:, 1:2], in_=coefs2.ap().rearrange("(p one) -> p one", one=1)
        )

        # knot-difference reciprocals, masked where the denominator is 0:
        # cols 0..2 = ld_k = knots[i+k]-knots[i], cols 3..5 = rd_k =
        # knots[i+k+1]-knots[i+1], k=1..3
        d6 = consts.tile([P, 6], DT)
        nc.vector.tensor_scalar(
            out=d6[:, 0:3], in0=kcolT[:, 1:4], scalar1=kcolT[:, 0:1],
            scalar2=None, op0=OP.subtract,
        )
        nc.vector.tensor_scalar(
            out=d6[:, 3:6], in0=kcolT[:, 2:5], scalar1=kcolT[:, 1:2],
            scalar2=None, op0=OP.subtract,
        )
        dmask = consts.tile([P, 6], DT)
        nc.vector.tensor_scalar(
            out=dmask, in0=d6, scalar1=0.0, scalar2=None, op0=OP.is_gt
        )
        # safe = max(d, eps): keeps valid denominators bit-exact (no
        # cancellation), makes the reciprocal finite on empty-span rows
        # (those are zeroed by dmask afterwards).
        dsafe = consts.tile([P, 6], DT)
        nc.vector.tensor_scalar(
            out=dsafe, in0=d6, scalar1=1e-6, scalar2=None, op0=OP.max
        )
        ihat = consts.tile([P, 6], DT)
        nc.vector.reciprocal(out=ihat, in_=dsafe)
        nc.vector.tensor_tensor(out=ihat, in0=ihat, in1=dmask, op=OP.mult)

        # v = (knots[i+k+1] - x)*ird = (x - knots[i+k+1]) * (-ird):
        # keep -ird so both u and v use the (subtract, mult) two-scalar form,
        # which avoids cancellation (x - knot is Sterbenz-exact).
        nird = consts.tile([P, 3], DT)
        nc.vector.tensor_scalar(
            out=nird, in0=ihat[:, 3:6], scalar1=-1.0, scalar2=None, op0=OP.mult
        )

        # subdiagonal shift matrix: shm[j, i] = 1 iff j == i+1, so that
        # (shm.T @ B)[i] = B[i+1] (row 127 -> 0)
        ones_t = consts.tile([P, P], DT)
        nc.gpsimd.memset(ones_t, 1.0)
        shm = consts.tile([P, P], DT)
        nc.gpsimd.affine_select(
            out=shm, in_=ones_t, pattern=[[-1, P]], base=-1,
            channel_multiplier=1, compare_op=OP.is_equal, fill=0.0,
        )

        # DRAM scratch for the spline rows: row 0 = sx (first nxs cols),
        # row 1 = st (last nt cols)
        srow_d = dram.tile([2, npts], DT)

        # ---- per-chunk basis recursion + contraction ----
        for ci in range(nchunks):
            g0 = ci * w
            src = xs.ap()[g0 : g0 + w] if g0 < nxs else tpts.ap()[g0 - nxs : g0 - nxs + w]
            xb = work.tile([P, w], DT, tag="xb")
            nc.gpsimd.dma_start(out=xb, in_=src.unsqueeze(0).to_broadcast([P, w]))

            bge = work.tile([P, w], DT, tag="bge")
            nc.vector.tensor_scalar(
                out=bge, in0=xb, scalar1=kcolT[:, 0:1], scalar2=None, op0=OP.is_ge
            )
            blt = work.tile([P, w], DT, tag="blt")
            nc.vector.tensor_scalar(
                out=blt, in0=xb, scalar1=kcolT[:, 1:2], scalar2=None, op0=OP.is_lt
            )
            b = work.tile([P, w], DT, tag="b0")
            nc.vector.tensor_tensor(out=b, in0=bge, in1=blt, op=OP.mult)

            for k in range(1, 4):
                u = work.tile([P, w], DT, tag="u")
                nc.vector.tensor_scalar(
                    out=u, in0=xb, scalar1=kcolT[:, 0:1],
                    scalar2=ihat[:, k - 1 : k], op0=OP.subtract, op1=OP.mult,
                )
                v = work.tile([P, w], DT, tag="v")
                nc.vector.tensor_scalar(
                    out=v, in0=xb, scalar1=kcolT[:, k + 1 : k + 2],
                    scalar2=nird[:, k - 1 : k], op0=OP.subtract, op1=OP.mult,
                )
                bs = psum.tile([P, w], DT, tag="bs")
                nc.tensor.matmul(bs, lhsT=shm, rhs=b, start=True, stop=True)
                t1 = work.tile([P, w], DT, tag="t1")
                nc.vector.tensor_tensor(out=t1, in0=u, in1=b, op=OP.mult)
                t2 = work.tile([P, w], DT, tag="t2")
                nc.vector.tensor_tensor(out=t2, in0=v, in1=bs, op=OP.mult)
                b = work.tile([P, w], DT, tag=f"b{k}")
                nc.gpsimd.tensor_tensor(out=b, in0=t1, in1=t2, op=OP.add)

            ps = psum_c.tile([2, w], DT, tag="contract")
            nc.tensor.matmul(ps, lhsT=ccol, rhs=b, start=True, stop=True)
            stg = work.tile([2, w], DT, tag="stg")
            nc.any.tensor_copy(out=stg, in_=ps)
            nc.sync.dma_start(out=srow_d[:, g0 : g0 + w], in_=stg)

        # ---- outer product ----
        # sx as a column tile: sxcol[p, r] = sx[r*128 + p]
        sxcol = consts.tile([P, n_row_tiles], DT)
        nc.sync.dma_start(
            out=sxcol,
            in_=srow_d[0:1, 0:nxs].rearrange("one (r p) -> p (one r)", p=P),
        )
        # st broadcast to all partitions
        stb = consts.tile([P, nt], DT)
        nc.gpsimd.dma_start(
            out=stb, in_=srow_d[1:2, nxs:npts].to_broadcast([P, nt])
        )

        for r in range(n_row_tiles):
            ot = outp.tile([P, nt], DT, tag="ot")
            if r % 2 == 0:
                nc.vector.tensor_scalar(
                    out=ot, in0=stb, scalar1=sxcol[:, r : r + 1], scalar2=None,
                    op0=OP.mult,
                )
            else:
                nc.scalar.activation(
                    out=ot, in_=stb, func=AF.Copy, scale=sxcol[:, r : r + 1]
                )
            nc.sync.dma_start(out=out.ap()[r * P : (r + 1) * P, :], in_=ot)


def emit_core_program_v2(nc, tc, ctx, xs, tpts, knots, coefs, coefs2, out, nxs, nt):
    """Table-based evaluation.

    The spline restricted to span j (j = 0..124, span = [knots[j+3],
    knots[j+4])) is a cubic. We build, on device, a per-span table
    T[j] = [kleft, invh, Ax, Bx, Cx, Dx, At, Bt, Ct, Dt] (the cubic in the
    normalized local coordinate v = (x - kleft)*invh for both coef vectors)
    by evaluating the dense Cox-de Boor recursion at 4 nodes per span
    (v = 0, 1/4, 1/2, 3/4) and applying the exact 4-point interpolation
    matrix. Per-point evaluation then is: step matrix C[i, pt] =
    (x >= knots[i+3]); gathered row = C^T @ deltaT (prefix-sum trick);
    Horner in v. All pointwise work runs in a points-on-partitions layout
    ([128, npts/128]) where it is nearly free.
    """
    npts = nxs + nt
    assert nxs % P == 0 and nt % P == 0
    nch = npts // P            # 128-point gather chunks
    nxch = nxs // P
    n_row_tiles = nxs // P
    GRP = 1024                 # xb broadcast group width
    assert nxs % GRP == 0 and nt % GRP == 0

    # Lagrange nodes in normalized local coordinate v, spanning [0, 1).
    # v=1 itself is excluded (the dense-CdB node evaluation at the right
    # knot of the LAST span would return 0, not the left limit), so the
    # last node sits just inside.
    NODES = [0.0, 1.0 / 3.0, 2.0 / 3.0, 1.0 - 1.0 / 4096.0]
    # barycentric-style scale: c_q = 1 / prod_{r != q} (v_q - v_r)
    LAGC = []
    for q in range(4):
        prod = 1.0
        for r in range(4):
            if r != q:
                prod *= NODES[q] - NODES[r]
        LAGC.append(1.0 / prod)

    if True:
        consts = ctx.enter_context(tc.tile_pool(name="consts", bufs=1))
        work = ctx.enter_context(tc.tile_pool(name="work", bufs=2))
        cpool = ctx.enter_context(tc.tile_pool(name="cpool", bufs=3))
        outp = ctx.enter_context(tc.tile_pool(name="outp", bufs=2))
        psum = ctx.enter_context(tc.tile_pool(name="psum", bufs=1, space="PSUM"))
        psum_g = ctx.enter_context(tc.tile_pool(name="psum_g", bufs=4, space="PSUM"))
        dram = ctx.enter_context(tc.tile_pool(name="dram", bufs=1, space="DRAM"))

        # ---- shared constants (same as V1) ----
        kcolT = consts.tile([P, 5], DT)
        nc.sync.dma_start(out=kcolT, in_=bass.AP(tensor=knots, offset=0, ap=[[1, P], [1, 5]]))
        ccol = consts.tile([P, 2], DT)
        nc.sync.dma_start(out=ccol[:, 0:1], in_=coefs.ap().rearrange("(p one) -> p one", one=1))
        nc.sync.dma_start(out=ccol[:, 1:2], in_=coefs2.ap().rearrange("(p one) -> p one", one=1))

        d6 = consts.tile([P, 6], DT)
        nc.vector.tensor_scalar(out=d6[:, 0:3], in0=kcolT[:, 1:4], scalar1=kcolT[:, 0:1], scalar2=None, op0=OP.subtract)
        nc.vector.tensor_scalar(out=d6[:, 3:6], in0=kcolT[:, 2:5], scalar1=kcolT[:, 1:2], scalar2=None, op0=OP.subtract)
        dmask = consts.tile([P, 6], DT)
        nc.vector.tensor_scalar(out=dmask, in0=d6, scalar1=0.0, scalar2=None, op0=OP.is_gt)
        dsafe = consts.tile([P, 6], DT)
        nc.vector.tensor_scalar(out=dsafe, in0=d6, scalar1=1e-6, scalar2=None, op0=OP.max)
        ihat = consts.tile([P, 6], DT)
        nc.vector.reciprocal(out=ihat, in_=dsafe)
        nc.vector.tensor_tensor(out=ihat, in0=ihat, in1=dmask, op=OP.mult)
        nird = consts.tile([P, 3], DT)
        nc.vector.tensor_scalar(out=nird, in0=ihat[:, 3:6], scalar1=-1.0, scalar2=None, op0=OP.mult)

        ones_t = consts.tile([P, P], DT)
        nc.gpsimd.memset(ones_t, 1.0)
        shm = consts.tile([P, P], DT)   # up-shift: (shm.T @ B)[i] = B[i+1]
        nc.gpsimd.affine_select(out=shm, in_=ones_t, pattern=[[-1, P]], base=-1, channel_multiplier=1, compare_op=OP.is_equal, fill=0.0)


        # ---- node construction: y[j, q] = knots[j+3] + v_q * h_j ----
        h_col = consts.tile([P, 1], DT)
        nc.vector.tensor_tensor(out=h_col, in0=kcolT[:, 4:5], in1=kcolT[:, 3:4], op=OP.subtract)
        ynod = consts.tile([P, 4], DT)
        for q in range(4):
            nc.vector.tensor_scalar(
                out=ynod[:, q : q + 1], in0=h_col, scalar1=float(NODES[q]),
                scalar2=kcolT[:, 3:4], op0=OP.mult, op1=OP.add,
            )

        ynod_d = dram.tile([P * 4], DT)
        nc.sync.dma_start(out=ynod_d.rearrange("(j q) -> j q", q=4), in_=ynod)

        # ---- dense CdB at the 512 nodes ----
        wn = P * 4
        xbn = consts.tile([P, wn], DT)
        nc.gpsimd.dma_start(out=xbn, in_=ynod_d.unsqueeze(0).to_broadcast([P, wn]))
        bge = consts.tile([P, wn], DT)
        nc.vector.tensor_scalar(out=bge, in0=xbn, scalar1=kcolT[:, 0:1], scalar2=None, op0=OP.is_ge)
        blt = consts.tile([P, wn], DT)
        nc.vector.tensor_scalar(out=blt, in0=xbn, scalar1=kcolT[:, 1:2], scalar2=None, op0=OP.is_lt)
        bnod = consts.tile([P, wn], DT)
        nc.vector.tensor_tensor(out=bnod, in0=bge, in1=blt, op=OP.mult)
        for k in range(1, 4):
            u = consts.tile([P, wn], DT, tag=f"nu{k}")
            nc.vector.tensor_scalar(out=u, in0=xbn, scalar1=kcolT[:, 0:1], scalar2=ihat[:, k - 1 : k], op0=OP.subtract, op1=OP.mult)
            v = consts.tile([P, wn], DT, tag=f"nv{k}")
            nc.vector.tensor_scalar(out=v, in0=xbn, scalar1=kcolT[:, k + 1 : k + 2], scalar2=nird[:, k - 1 : k], op0=OP.subtract, op1=OP.mult)
            bsp = psum.tile([P, wn], DT, tag="nshift")
            nc.tensor.matmul(bsp, lhsT=shm, rhs=bnod, start=True, stop=True)
            t1 = consts.tile([P, wn], DT, tag=f"nt1{k}")
            nc.vector.tensor_tensor(out=t1, in0=u, in1=bnod, op=OP.mult)
            t2 = consts.tile([P, wn], DT, tag=f"nt2{k}")
            nc.vector.tensor_tensor(out=t2, in0=v, in1=bsp, op=OP.mult)
            bnod = consts.tile([P, wn], DT, tag=f"nb{k}")
            nc.vector.tensor_tensor(out=bnod, in0=t1, in1=t2, op=OP.add)

        svp = psum.tile([2, wn], DT, tag="ncontract")
        nc.tensor.matmul(svp, lhsT=ccol, rhs=bnod, start=True, stop=True)
        sv = consts.tile([2, wn], DT)
        nc.vector.tensor_copy(out=sv, in_=svp)
        svd = dram.tile([2, wn], DT)
        nc.sync.dma_start(out=svd, in_=sv)

        # node values per span: svt[j, c, q] = sv[c, j*4+q]
        svt = consts.tile([P, 2, 4], DT)
        nc.sync.dma_start(
            out=svt,
            in_=bass.AP(tensor=svd.tensor, offset=svd.offset, ap=[[4, P], [wn, 2], [1, 4]]),
        )

        # ---- table: T = [kleft, invh, c_q*svx_q (q=0..3), c_q*svt_q] ----
        tt_ = consts.tile([P, 10], DT)
        nc.vector.tensor_copy(out=tt_[:, 0:1], in_=kcolT[:, 3:4])
        hsafe = consts.tile([P, 1], DT)
        nc.vector.tensor_scalar(out=hsafe, in0=h_col, scalar1=1e-6, scalar2=None, op0=OP.max)
        hrec = consts.tile([P, 1], DT)
        nc.vector.reciprocal(out=hrec, in_=hsafe)
        hmask = consts.tile([P, 1], DT)
        nc.vector.tensor_scalar(out=hmask, in0=h_col, scalar1=0.0, scalar2=None, op0=OP.is_gt)
        nc.vector.tensor_tensor(out=tt_[:, 1:2], in0=hrec, in1=hmask, op=OP.mult)

        for c in range(2):
            for q in range(4):
                nc.vector.tensor_scalar(
                    out=tt_[:, 2 + c * 4 + q : 3 + c * 4 + q],
                    in0=svt[:, c, q : q + 1], scalar1=float(LAGC[q]),
                    scalar2=None, op0=OP.mult,
                )

        # ---- per-point gather via exact one-hot (no prefix-sum rounding):
        # onehot[i, pt] = (knots[i+3] <= x_pt) * (x_pt < knots[i+4]),
        # gathered row = onehot^T @ T (single 1.0 * T[span] product).
        gall = consts.tile([P, nch * 10], DT)
        for gi in range(npts // GRP):
            g0 = gi * GRP
            src = xs.ap()[g0 : g0 + GRP] if g0 < nxs else tpts.ap()[g0 - nxs : g0 - nxs + GRP]
            xbg = work.tile([P, GRP], DT, tag="xbg")
            nc.gpsimd.dma_start(out=xbg, in_=src.unsqueeze(0).to_broadcast([P, GRP]))
            for cj in range(GRP // P):
                ci = gi * (GRP // P) + cj
                xbc = xbg[:, cj * P : (cj + 1) * P]
                cge = cpool.tile([P, P], DT, tag="cge")
                nc.vector.tensor_scalar(
                    out=cge, in0=xbc, scalar1=kcolT[:, 3:4], scalar2=None, op0=OP.is_ge
                )
                clt = cpool.tile([P, P], DT, tag="clt")
                nc.vector.tensor_scalar(
                    out=clt, in0=xbc, scalar1=kcolT[:, 4:5], scalar2=None, op0=OP.is_lt
                )
                cmat = cpool.tile([P, P], DT, tag="cmat")
                nc.vector.tensor_tensor(out=cmat, in0=cge, in1=clt, op=OP.mult)
                psg = psum_g.tile([P, 10], DT, tag="gather")
                nc.tensor.matmul(psg, lhsT=cmat, rhs=tt_, start=True, stop=True)
                if ci % 2 == 0:
                    nc.vector.tensor_copy(out=gall[:, ci * 10 : (ci + 1) * 10], in_=psg)
                else:
                    nc.scalar.copy(out=gall[:, ci * 10 : (ci + 1) * 10], in_=psg)

        # ---- pointwise Horner in v = (x - kleft) * invh ----
        xcol = consts.tile([P, nch], DT)
        nc.sync.dma_start(out=xcol[:, 0:nxch], in_=xs.ap().rearrange("(c p) -> p c", p=P))
        nc.sync.dma_start(out=xcol[:, nxch:nch], in_=tpts.ap().rearrange("(c p) -> p c", p=P))

        gv = gall.rearrange("p (c ten) -> p c ten", ten=10)
        wloc = consts.tile([P, nch], DT)
        nc.vector.tensor_tensor(out=wloc, in0=xcol, in1=gv[:, :, 0], op=OP.subtract)
        vloc = consts.tile([P, nch], DT)
        nc.vector.tensor_tensor(out=vloc, in0=wloc, in1=gv[:, :, 1], op=OP.mult)

        # Lagrange basis products: d_q = v - v_q; L0' = d1*d2*d3, etc.
        dq = [consts.tile([P, nch], DT, tag=f"dq{q}", name=f"dq{q}") for q in range(4)]
        for q in range(4):
            nc.vector.tensor_scalar(
                out=dq[q], in0=vloc, scalar1=float(NODES[q]), scalar2=None,
                op0=OP.subtract,
            )
        d01 = consts.tile([P, nch], DT)
        nc.vector.tensor_tensor(out=d01, in0=dq[0], in1=dq[1], op=OP.mult)
        d23 = consts.tile([P, nch], DT)
        nc.vector.tensor_tensor(out=d23, in0=dq[2], in1=dq[3], op=OP.mult)
        lag = [consts.tile([P, nch], DT, tag=f"lag{q}", name=f"lag{q}") for q in range(4)]
        nc.vector.tensor_tensor(out=lag[0], in0=dq[1], in1=d23, op=OP.mult)
        nc.vector.tensor_tensor(out=lag[1], in0=dq[0], in1=d23, op=OP.mult)
        nc.vector.tensor_tensor(out=lag[2], in0=dq[3], in1=d01, op=OP.mult)
        nc.vector.tensor_tensor(out=lag[3], in0=dq[2], in1=d01, op=OP.mult)

        # s = sum_q L_q' * (c_q * sv_q)  (scale folded into the table)
        sres = consts.tile([P, nch], DT)
        for part, (c0, c1) in enumerate([(0, nxch), (nxch, nch)]):
            base = 2 + part * 4
            acc = consts.tile([P, c1 - c0], DT, tag=f"acc{part}")
            t0_ = consts.tile([P, c1 - c0], DT, tag=f"t0_{part}")
            nc.vector.tensor_tensor(out=acc, in0=lag[0][:, c0:c1], in1=gv[:, c0:c1, base + 0], op=OP.mult)
            for q in range(1, 4):
                nc.vector.tensor_tensor(out=t0_, in0=lag[q][:, c0:c1], in1=gv[:, c0:c1, base + q], op=OP.mult)
                nc.vector.tensor_tensor(out=acc, in0=acc, in1=t0_, op=OP.add)
            nc.vector.tensor_copy(out=sres[:, c0:c1], in_=acc)

        sxcol = sres[:, 0:nxch]          # [128, 8] column layout, ready for output
        strow_d = dram.tile([nt], DT)
        nc.sync.dma_start(out=strow_d.rearrange("(c p) -> p c", p=P), in_=sres[:, nxch:nch])

        stb = consts.tile([P, nt], DT)
        nc.gpsimd.dma_start(out=stb, in_=strow_d.unsqueeze(0).to_broadcast([P, nt]))

        # ---- outer product + output ----
        for r in range(n_row_tiles):
            ot = outp.tile([P, nt], DT, tag="ot")
            eng = r % 3
            if eng == 0:
                nc.vector.tensor_scalar(out=ot, in0=stb, scalar1=sxcol[:, r : r + 1], scalar2=None, op0=OP.mult)
            elif eng == 1:
                nc.scalar.activation(out=ot, in_=stb, func=AF.Copy, scale=sxcol[:, r : r + 1])
            else:
                nc.gpsimd.tensor_scalar(out=ot, in0=stb, scalar1=sxcol[:, r : r + 1], scalar2=None, op0=OP.mult)
            nc.sync.dma_start(out=out.ap()[r * P : (r + 1) * P, :], in_=ot)


def emit_core_program_v3(nc, tc, ctx, xs, tpts, knots, coefs, coefs2, out, nxs, nt):
    """V2 with batched wide ops (overhead-bound fix): one-hot C built in
    2304-wide quarters, gather matmuls back-to-back, PSUM extracts batched
    4 chunks per copy."""
    npts = nxs + nt
    assert nxs % P == 0 and nt % P == 0
    nch = npts // P
    nxch = nxs // P
    n_row_tiles = nxs // P
    QW = 1536                  # slab width (points); 1536 = 3 * 512
    assert npts % QW == 0 and QW % P == 0

    NODES = [0.0, 1.0 / 3.0, 2.0 / 3.0, 1.0 - 1.0 / 4096.0]
    LAGC = []
    for q in range(4):
        prod = 1.0
        for r in range(4):
            if r != q:
                prod *= NODES[q] - NODES[r]
        LAGC.append(1.0 / prod)

    if True:
        consts = ctx.enter_context(tc.tile_pool(name="consts", bufs=1))
        work = ctx.enter_context(tc.tile_pool(name="work", bufs=2))
        outp = ctx.enter_context(tc.tile_pool(name="outp", bufs=2))
        psum = ctx.enter_context(tc.tile_pool(name="psum", bufs=1, space="PSUM"))
        psum_g = ctx.enter_context(tc.tile_pool(name="psum_g", bufs=4, space="PSUM"))
        dram = ctx.enter_context(tc.tile_pool(name="dram", bufs=1, space="DRAM"))

        # ---- constants ----
        kcolT = consts.tile([P, 5], DT)
        nc.sync.dma_start(out=kcolT, in_=bass.AP(tensor=knots, offset=0, ap=[[1, P], [1, 5]]))
        ccol = consts.tile([P, 2], DT)
        nc.sync.dma_start(out=ccol[:, 0:1], in_=coefs.ap().rearrange("(p one) -> p one", one=1))
        nc.sync.dma_start(out=ccol[:, 1:2], in_=coefs2.ap().rearrange("(p one) -> p one", one=1))

        d6 = consts.tile([P, 6], DT)
        nc.vector.tensor_scalar(out=d6[:, 0:3], in0=kcolT[:, 1:4], scalar1=kcolT[:, 0:1], scalar2=None, op0=OP.subtract)
        nc.vector.tensor_scalar(out=d6[:, 3:6], in0=kcolT[:, 2:5], scalar1=kcolT[:, 1:2], scalar2=None, op0=OP.subtract)
        dmask = consts.tile([P, 6], DT)
        nc.vector.tensor_scalar(out=dmask, in0=d6, scalar1=0.0, scalar2=None, op0=OP.is_gt)
        dsafe = consts.tile([P, 6], DT)
        nc.vector.tensor_scalar(out=dsafe, in0=d6, scalar1=1e-6, scalar2=None, op0=OP.max)
        ihat = consts.tile([P, 6], DT)
        nc.vector.reciprocal(out=ihat, in_=dsafe)
        nc.vector.tensor_tensor(out=ihat, in0=ihat, in1=dmask, op=OP.mult)
        nird = consts.tile([P, 3], DT)
        nc.vector.tensor_scalar(out=nird, in0=ihat[:, 3:6], scalar1=-1.0, scalar2=None, op0=OP.mult)

        ones_t = consts.tile([P, P], DT)
        nc.gpsimd.memset(ones_t, 1.0)
        shm = consts.tile([P, P], DT)
        nc.gpsimd.affine_select(out=shm, in_=ones_t, pattern=[[-1, P]], base=-1, channel_multiplier=1, compare_op=OP.is_equal, fill=0.0)

        # ---- node construction ----
        h_col = consts.tile([P, 1], DT)
        nc.vector.tensor_tensor(out=h_col, in0=kcolT[:, 4:5], in1=kcolT[:, 3:4], op=OP.subtract)
        ynod = consts.tile([P, 4], DT)
        for q in range(4):
            nc.vector.tensor_scalar(
                out=ynod[:, q : q + 1], in0=h_col, scalar1=float(NODES[q]),
                scalar2=kcolT[:, 3:4], op0=OP.mult, op1=OP.add,
            )
        ynod_d = dram.tile([P * 4], DT)
        nc.sync.dma_start(out=ynod_d.rearrange("(j q) -> j q", q=4), in_=ynod)

        # ---- dense CdB at the 512 nodes ----
        wn = P * 4
        xbn = consts.tile([P, wn], DT)
        nc.gpsimd.dma_start(out=xbn, in_=ynod_d.unsqueeze(0).to_broadcast([P, wn]))
        bge = consts.tile([P, wn], DT)
        nc.vector.tensor_scalar(out=bge, in0=xbn, scalar1=kcolT[:, 0:1], scalar2=None, op0=OP.is_ge)
        blt = consts.tile([P, wn], DT)
        nc.vector.tensor_scalar(out=blt, in0=xbn, scalar1=kcolT[:, 1:2], scalar2=None, op0=OP.is_lt)
        bnod = consts.tile([P, wn], DT)
        nc.vector.tensor_tensor(out=bnod, in0=bge, in1=blt, op=OP.mult)
        for k in range(1, 4):
            u = consts.tile([P, wn], DT, tag=f"nu{k}", name=f"nu{k}")
            nc.vector.tensor_scalar(out=u, in0=xbn, scalar1=kcolT[:, 0:1], scalar2=ihat[:, k - 1 : k], op0=OP.subtract, op1=OP.mult)
            v = consts.tile([P, wn], DT, tag=f"nv{k}", name=f"nv{k}")
            nc.vector.tensor_scalar(out=v, in0=xbn, scalar1=kcolT[:, k + 1 : k + 2], scalar2=nird[:, k - 1 : k], op0=OP.subtract, op1=OP.mult)
            bsp = psum.tile([P, wn], DT, tag="nshift", name=f"nshift{k}")
            nc.tensor.matmul(bsp, lhsT=shm, rhs=bnod, start=True, stop=True)
            t1 = consts.tile([P, wn], DT, tag=f"nt1{k}", name=f"nt1{k}")
            nc.vector.tensor_tensor(out=t1, in0=u, in1=bnod, op=OP.mult)
            t2 = consts.tile([P, wn], DT, tag=f"nt2{k}", name=f"nt2{k}")
            nc.vector.tensor_tensor(out=t2, in0=v, in1=bsp, op=OP.mult)
            bnod = consts.tile([P, wn], DT, tag=f"nb{k}", name=f"nb{k}")
            nc.vector.tensor_tensor(out=bnod, in0=t1, in1=t2, op=OP.add)

        svp = psum.tile([2, wn], DT, tag="ncontract")
        nc.tensor.matmul(svp, lhsT=ccol, rhs=bnod, start=True, stop=True)
        sv = consts.tile([2, wn], DT)
        nc.vector.tensor_copy(out=sv, in_=svp)
        svd = dram.tile([2, wn], DT)
        nc.sync.dma_start(out=svd, in_=sv)
        svt = consts.tile([P, 2, 4], DT)
        nc.sync.dma_start(
            out=svt,
            in_=bass.AP(tensor=svd.tensor, offset=svd.offset, ap=[[4, P], [wn, 2], [1, 4]]),
        )

        # ---- table ----
        tt_ = consts.tile([P, 10], DT)
        nc.vector.tensor_copy(out=tt_[:, 0:1], in_=kcolT[:, 3:4])
        hsafe = consts.tile([P, 1], DT)
        nc.vector.tensor_scalar(out=hsafe, in0=h_col, scalar1=1e-6, scalar2=None, op0=OP.max)
        hrec = consts.tile([P, 1], DT)
        nc.vector.reciprocal(out=hrec, in_=hsafe)
        hmask = consts.tile([P, 1], DT)
        nc.vector.tensor_scalar(out=hmask, in0=h_col, scalar1=0.0, scalar2=None, op0=OP.is_gt)
        nc.vector.tensor_tensor(out=tt_[:, 1:2], in0=hrec, in1=hmask, op=OP.mult)
        for c in range(2):
            for q in range(4):
                nc.vector.tensor_scalar(
                    out=tt_[:, 2 + c * 4 + q : 3 + c * 4 + q],
                    in0=svt[:, c, q : q + 1], scalar1=float(LAGC[q]),
                    scalar2=None, op0=OP.mult,
                )

        # ---- batched one-hot + gather ----
        gall = consts.tile([P, nch * 10], DT)
        nq = npts // QW
        for qi in range(nq):
            g0 = qi * QW
            xbq = work.tile([P, QW], DT, tag="xbq")
            if g0 + QW <= nxs:
                nc.gpsimd.dma_start(out=xbq, in_=xs.ap()[g0 : g0 + QW].unsqueeze(0).to_broadcast([P, QW]))
            elif g0 >= nxs:
                nc.gpsimd.dma_start(out=xbq, in_=tpts.ap()[g0 - nxs : g0 - nxs + QW].unsqueeze(0).to_broadcast([P, QW]))
            else:
                nx_part = nxs - g0
                nc.gpsimd.dma_start(out=xbq[:, 0:nx_part], in_=xs.ap()[g0:nxs].unsqueeze(0).to_broadcast([P, nx_part]))
                nc.gpsimd.dma_start(out=xbq[:, nx_part:QW], in_=tpts.ap()[0 : QW - nx_part].unsqueeze(0).to_broadcast([P, QW - nx_part]))
            cge = work.tile([P, QW], DT, tag="cge")
            nc.vector.tensor_scalar(out=cge, in0=xbq, scalar1=kcolT[:, 3:4], scalar2=None, op0=OP.is_ge)
            clt = work.tile([P, QW], DT, tag="clt")
            nc.vector.tensor_scalar(out=clt, in0=xbq, scalar1=kcolT[:, 4:5], scalar2=None, op0=OP.is_lt)
            cmat = work.tile([P, QW], DT, tag="cmat")
            nc.vector.tensor_tensor(out=cmat, in0=cge, in1=clt, op=OP.mult)

            nblk = QW // (4 * P)   # 4 gather chunks per PSUM extract
            for b in range(nblk):
                psg = psum_g.tile([P, 40], DT, tag="gather")
                for j in range(4):
                    cj = b * 4 + j
                    nc.tensor.matmul(
                        psg[:, j * 10 : (j + 1) * 10],
                        lhsT=cmat[:, (cj * P) : (cj + 1) * P],
                        rhs=tt_, start=True, stop=True,
                    )
                ci0 = qi * (QW // P) + b * 4
                if b % 2 == 0:
                    nc.vector.tensor_copy(out=gall[:, ci0 * 10 : (ci0 + 4) * 10], in_=psg)
                else:
                    nc.scalar.copy(out=gall[:, ci0 * 10 : (ci0 + 4) * 10], in_=psg)

        # ---- pointwise Lagrange ----
        xcol = consts.tile([P, nch], DT)
        nc.sync.dma_start(out=xcol[:, 0:nxch], in_=xs.ap().rearrange("(c p) -> p c", p=P))
        nc.sync.dma_start(out=xcol[:, nxch:nch], in_=tpts.ap().rearrange("(c p) -> p c", p=P))

        gv = gall.rearrange("p (c ten) -> p c ten", ten=10)
        wloc = consts.tile([P, nch], DT)
        nc.vector.tensor_tensor(out=wloc, in0=xcol, in1=gv[:, :, 0], op=OP.subtract)
        vloc = consts.tile([P, nch], DT)
        nc.vector.tensor_tensor(out=vloc, in0=wloc, in1=gv[:, :, 1], op=OP.mult)
        dq = [consts.tile([P, nch], DT, tag=f"dq{q}", name=f"dq{q}") for q in range(4)]
        for q in range(4):
            nc.vector.tensor_scalar(out=dq[q], in0=vloc, scalar1=float(NODES[q]), scalar2=None, op0=OP.subtract)
        d01 = consts.tile([P, nch], DT)
        nc.vector.tensor_tensor(out=d01, in0=dq[0], in1=dq[1], op=OP.mult)
        d23 = consts.tile([P, nch], DT)
        nc.vector.tensor_tensor(out=d23, in0=dq[2], in1=dq[3], op=OP.mult)
        lag = [consts.tile([P, nch], DT, tag=f"lag{q}", name=f"lag{q}") for q in range(4)]
        nc.vector.tensor_tensor(out=lag[0], in0=dq[1], in1=d23, op=OP.mult)
        nc.vector.tensor_tensor(out=lag[1], in0=dq[0], in1=d23, op=OP.mult)
        nc.vector.tensor_tensor(out=lag[2], in0=dq[3], in1=d01, op=OP.mult)
        nc.vector.tensor_tensor(out=lag[3], in0=dq[2], in1=d01, op=OP.mult)

        sres = consts.tile([P, nch], DT)
        for part, (c0, c1) in enumerate([(0, nxch), (nxch, nch)]):
            base = 2 + part * 4
            acc = consts.tile([P, c1 - c0], DT, tag=f"acc{part}", name=f"acc{part}")
            t0_ = consts.tile([P, c1 - c0], DT, tag=f"t0_{part}", name=f"t0_{part}")
            nc.vector.tensor_tensor(out=acc, in0=lag[0][:, c0:c1], in1=gv[:, c0:c1, base + 0], op=OP.mult)
            for q in range(1, 4):
                nc.vector.tensor_tensor(out=t0_, in0=lag[q][:, c0:c1], in1=gv[:, c0:c1, base + q], op=OP.mult)
                nc.vector.tensor_tensor(out=acc, in0=acc, in1=t0_, op=OP.add)
            nc.vector.tensor_copy(out=sres[:, c0:c1], in_=acc)

        sxcol = sres[:, 0:nxch]
        strow_d = dram.tile([nt], DT)
        nc.sync.dma_start(out=strow_d.rearrange("(c p) -> p c", p=P), in_=sres[:, nxch:nch])
        stb = consts.tile([P, nt], DT)
        nc.gpsimd.dma_start(out=stb, in_=strow_d.unsqueeze(0).to_broadcast([P, nt]))

        # ---- outer product + output ----
        for r in range(n_row_tiles):
            ot = outp.tile([P, nt], DT, tag="ot")
            eng = r % 3
            if eng == 0:
                nc.vector.tensor_scalar(out=ot, in0=stb, scalar1=sxcol[:, r : r + 1], scalar2=None, op0=OP.mult)
            elif eng == 1:
                nc.scalar.activation(out=ot, in_=stb, func=AF.Copy, scale=sxcol[:, r : r + 1])
            else:
                nc.gpsimd.tensor_scalar(out=ot, in0=stb, scalar1=sxcol[:, r : r + 1], scalar2=None, op0=OP.mult)
            nc.sync.dma_start(out=out.ap()[r * P : (r + 1) * P, :], in_=ot)



def build_program(nxs=NX // N_CORES, nt=NT, w=512, debug=False, version=1,
                  unroll=1):
    """Build the per-core Bacc program. unroll>1 emits the body N times
    (used for marginal-difference timing)."""
    nc = bacc.Bacc("TRN2", target_bir_lowering=False, debug=debug)
    xs = nc.dram_tensor("xs", [nxs], DT, kind="ExternalInput")
    tpts = nc.dram_tensor("t", [nt], DT, kind="ExternalInput")
    knots = nc.dram_tensor("knots", [NK], DT, kind="ExternalInput")
    coefs = nc.dram_tensor("coefs", [NCF], DT, kind="ExternalInput")
    coefs2 = nc.dram_tensor("coefs2", [NCF], DT, kind="ExternalInput")
    out = nc.dram_tensor("out", [nxs, nt], DT, kind="ExternalOutput")

    emit = {1: emit_core_program, 2: emit_core_program_v2, 3: emit_core_program_v3}[version]
    kw = {"w": w} if version == 1 else {}

    with tile.TileContext(nc) as tc, ExitStack() as ctx:
        if unroll == 1:
            emit(nc, tc, ctx, xs, tpts, knots, coefs, coefs2, out, nxs, nt, **kw)
        else:
            for _ in range(unroll):
                with ExitStack() as bctx:
                    emit(nc, tc, bctx, xs, tpts, knots, coefs, coefs2, out, nxs, nt, **kw)
    nc.compile()
    return nc


VERSION = 3

_RUNNER_CACHE = {}


class _Runner:
    """Persistent PJRT runner: jit-compiled once, reusable across calls."""

    def __init__(self, version, unroll=1):
        import jax
        from jax.sharding import Mesh, PartitionSpec
        from jax.experimental.shard_map import shard_map
        from concourse import bass2jax, mybir as mb

        bass2jax.install_neuronx_cc_hook()
        nc = build_program(version=version, unroll=unroll)
        self.nc = nc

        partition_name = (
            nc.partition_id_tensor.name if nc.partition_id_tensor else None
        )
        in_names, out_names, out_avals, zero_shapes = [], [], [], []
        for alloc in nc.m.functions[0].allocations:
            if not isinstance(alloc, mb.MemoryLocationSet):
                continue
            name = alloc.memorylocations[0].name
            if alloc.kind == "ExternalInput":
                if name != partition_name:
                    in_names.append(name)
            elif alloc.kind == "ExternalOutput":
                shape = tuple(alloc.tensor_shape)
                dtype = mb.dt.np(alloc.dtype)
                out_names.append(name)
                out_avals.append(jax.core.ShapedArray(shape, dtype))
                zero_shapes.append((shape, dtype))
        self.in_names = list(in_names)
        self.out_names = out_names
        self.zero_shapes = zero_shapes
        n_params = len(in_names)
        all_in_names = in_names + out_names
        if partition_name is not None:
            all_in_names = all_in_names + [partition_name]

        def _body(*args):
            operands = list(args)
            if partition_name is not None:
                operands.append(bass2jax.partition_id_tensor())
            outs = bass2jax._bass_exec_p.bind(
                *operands,
                out_avals=tuple(out_avals),
                in_names=tuple(all_in_names),
                out_names=tuple(out_names),
                lowering_input_output_aliases=(),
                sim_require_finite=True,
                sim_require_nnan=True,
                nc=nc,
            )
            return tuple(outs)

        devices = jax.devices()[:N_CORES]
        self.mesh = Mesh(np.asarray(devices), ("core",))
        n_outs = len(out_names)
        in_specs = (PartitionSpec("core"),) * (n_params + n_outs)
        out_specs = (PartitionSpec("core"),) * n_outs
        self.donate = tuple(range(n_params, n_params + n_outs))
        self.sharded = jax.jit(
            shard_map(_body, mesh=self.mesh, in_specs=in_specs,
                      out_specs=out_specs, check_rep=False),
            donate_argnums=self.donate,
            keep_unused=True,
        )
        self._jax = jax
        self._P = PartitionSpec

    def make_zeros(self):
        """Fresh donated output buffers, created on-device."""
        import jax
        import jax.numpy as jnp
        from jax.sharding import NamedSharding

        outs = []
        for shape, dtype in self.zero_shapes:
            gshape = (shape[0] * N_CORES,) + tuple(shape[1:])
            sharding = NamedSharding(self.mesh, self._P("core"))
            outs.append(jax.jit(
                lambda s=gshape, d=dtype: jnp.zeros(s, d),
                out_shardings=sharding)())
        return outs

    def run_device(self, concat_ins, zeros):
        """Returns device arrays (not transferred)."""
        return self.sharded(*concat_ins, *zeros)

    def run(self, in_maps):
        concat_ins = [
            np.concatenate([np.asarray(m[name]) for m in in_maps], axis=0)
            for name in self.in_names
        ]
        outs = self.run_device(concat_ins, self.make_zeros())
        return [np.asarray(o) for o in outs]


def _get_runner(version=None, unroll=1):
    key = (version or VERSION, unroll)
    if key not in _RUNNER_CACHE:
        _RUNNER_CACHE[key] = _Runner(key[0], unroll=key[1])
    return _RUNNER_CACHE[key]


def kernel(x, t, knots, coefs, coefs_2):
    x = np.ascontiguousarray(np.asarray(x, dtype=np.float32))
    t = np.ascontiguousarray(np.asarray(t, dtype=np.float32))
    knots = np.ascontiguousarray(np.asarray(knots, dtype=np.float32))
    coefs = np.ascontiguousarray(np.asarray(coefs, dtype=np.float32))
    coefs_2 = np.ascontiguousarray(np.asarray(coefs_2, dtype=np.float32))

    nxs = NX // N_CORES
    runner = _get_runner()
    in_maps = [
        {
            "xs": x[c * nxs : (c + 1) * nxs],
            "t": t,
            "knots": knots,
            "coefs": coefs,
            "coefs2": coefs_2,
        }
        for c in range(N_CORES)
    ]
    outs = runner.run(in_maps)
    return outs[runner.out_names.index("out")].reshape(NX, NT)
